# revision 37
# baseline (speedup 1.0000x reference)
"""GATv2 (2-layer, 4+1 heads) TRN2 bass kernel, 8-core SPMD.

Accepts FULL inputs as produced by reference.setup_inputs() and returns the
FULL [64, 2] output. Internally: edges are partitioned by destination core
(6250 nodes/core) and destination 128-node window, padded to a static
(TA, TB) tile schedule; per-edge messages are assembled in PSUM (edge-attr
and one-hot-expanded x_r matmuls; gathered x_l joins via a third matmul in
layer 1 and a batched DVE add in layer 2); leaky-relu is evaluated as
0.6*linear + 0.4*|m| with the linear logit part riding extra matmul columns;
softmax runs max-free (logits are bounded); aggregation uses onehot-matmul
scatter into per-window PSUM accumulators, with the gather-free self-loop
subtile computed first so it stays off the window critical path, and the
1/den normalization + relu folded into per-head scalar-engine activations.
The one-hot scatter tables (oh / ohT) and the self-loop mean edge-attr table
are precomputed on the host and streamed from HBM; stage-0 stores coalesce
to 4KB descriptors and the A-stream gathers depend only on the A half of
tab1 so they start mid-stage-0. Layer-2 tables are exchanged with an
on-device AllGather. Per-core pooled partials [64, 33] are combined on the
host with the final [32x2] classifier.
"""
import sys
for _p in ('/opt/trn_rl_repo', '/root/.axon_site/_ro/trn_rl_repo'):
    if _p not in sys.path:
        sys.path.insert(0, _p)

import numpy as np
import ml_dtypes

import concourse.bass as bass
import concourse.bacc as bacc
import concourse.mybir as mybir
import concourse.tile as tile

bf16 = ml_dtypes.bfloat16
AF = mybir.ActivationFunctionType
ALU = mybir.AluOpType
AX = mybir.AxisListType
DT = mybir.dt
NEG = 0.2
EPS = 1e-16
L2_DVE_ADD = True


class Cfg:
    def __init__(self, NC=8, VPCr=6250, TA=9, TB=9, G=64):
        self.NC = NC
        self.VPCr = VPCr
        self.N = NC * VPCr
        self.W = (VPCr + 127) // 128
        self.VPC = self.W * 128
        self.NPAD = NC * self.VPC
        self.TA, self.TB = TA, TB
        self.T = TA + TB
        self.G = G
        self.HALF = self.N // 2
        self.HALF2 = (NC // 2) * self.VPC
        self.HC = 128
        self.HEADS = 4
        self.CH = 32
        self.HID = 32
        self.SA = self.W * TA
        self.SB = self.W * TB
        self.CPC = max(d for d in (7, 3, 1)
                       if self.SA % d == 0 and self.SB % d == 0)
        self.NCALLA = self.SA // self.CPC
        self.NCALLB = self.SB // self.CPC
        self.NIDX = self.CPC * 128
        assert self.T % 3 == 0
        assert NC % 2 == 0 and VPCr % 2 == 0


def build_program(c: Cfg, debug=False, reps=1, skip_collective=False, ablate=()):
    nc = bacc.Bacc("TRN2", target_bir_lowering=False, debug=debug,
                   num_swdge_queues=4)
    f32, b16, i16 = DT.float32, DT.bfloat16, DT.int16

    def inp(name, shape, dt=f32):
        return nc.dram_tensor(name, shape, dt, kind="ExternalInput")

    xT = inp("xT", [128, c.NPAD], b16)
    xTloc = inp("xTloc", [128, c.VPC], b16)
    Wlr1x = inp("Wlr1x", [128, 264], b16)
    We1Q = inp("We1Q", [16, 132], b16)
    We2Q = inp("We2Q", [16, 33], b16)
    Wlr2x = inp("Wlr2x", [128, 66], b16)
    att1b = inp("att1b", [128, 396], b16)
    att2b = inp("att2b", [128, 297], b16)
    eye_bf = inp("eye_bf", [128, 128], b16)
    onescol = inp("onescol", [128, 1], b16)
    iota64 = inp("iota64", [128, 64], f32)
    eaT = inp("eaT", [16, c.W * c.T * 128], b16)
    ohW = inp("ohW", [128, c.W * c.T * 128], b16)
    ohTW = inp("ohTW", [128, c.W * c.T * 128], b16)
    laTh = inp("laTh", [16, c.W * 128], b16)
    NW = c.NIDX // 16
    idx1A = inp("idx1A", [128, c.NCALLA * NW], i16)
    idx1B = inp("idx1B", [128, c.NCALLB * NW], i16)
    idx2A = inp("idx2A", [128, c.NCALLA * NW], i16)
    idx2B = inp("idx2B", [128, c.NCALLB * NW], i16)
    batchloc = inp("batchloc", [128, c.W], f32)

    partial = nc.dram_tensor("partial", [64, 33], f32, kind="ExternalOutput")

    with tile.TileContext(nc) as tc:
        with (
            tc.tile_pool(name="const", bufs=1) as P_const,
            tc.tile_pool(name="res", bufs=1) as P_res,
            tc.tile_pool(name="s0", bufs=2) as P_s0,
            tc.tile_pool(name="gat", bufs=6) as P_gat,
            tc.tile_pool(name="ew", bufs=3) as P_ew,
            tc.tile_pool(name="mask", bufs=3) as P_mask,
            tc.tile_pool(name="grp", bufs=3) as P_grp,
            tc.tile_pool(name="win", bufs=2) as P_win,
            tc.tile_pool(name="pm", bufs=3, space="PSUM") as PS_m,
            tc.tile_pool(name="pnd", bufs=2, space="PSUM") as PS_nd,
            tc.tile_pool(name="pmisc", bufs=2, space="PSUM") as PS_misc,
            tc.tile_pool(name="ppool", bufs=1, space="PSUM") as PS_pool,
            tc.tile_pool(name="dram", bufs=1, space="DRAM") as P_dram,
        ):
            def load_const(t, shape, dt):
                s = P_const.tile(shape, dt, tag=t.name)
                nc.sync.dma_start(out=s[:], in_=t[:, :])
                return s

            Wlr1x_s = load_const(Wlr1x, [128, 264], b16)
            We1Q_s = load_const(We1Q, [16, 132], b16)
            We2Q_s = load_const(We2Q, [16, 33], b16)
            Wlr2x_s = load_const(Wlr2x, [128, 66], b16)
            att1b_s = load_const(att1b, [128, 396], b16)
            att2b_s = load_const(att2b, [128, 297], b16)
            eye_s = load_const(eye_bf, [128, 128], b16)
            onescol_s = load_const(onescol, [128, 1], b16)
            iota64_s = load_const(iota64, [128, 64], f32)
            laTh_s = load_const(laTh, [16, c.W * 128], b16)
            idx1A_s = load_const(idx1A, [128, c.NCALLA * NW], i16)
            idx1B_s = load_const(idx1B, [128, c.NCALLB * NW], i16)
            idx2A_s = load_const(idx2A, [128, c.NCALLA * NW], i16)
            idx2B_s = load_const(idx2B, [128, c.NCALLB * NW], i16)
            batchloc_s = load_const(batchloc, [128, c.W], f32)

            xl1c = P_res.tile([128, c.W * 132], b16)
            xr1c = P_res.tile([128, c.W * 132], b16)
            x2c = P_res.tile([128, c.W * 66], b16)

            tab1 = P_dram.tile([c.NPAD, 256], b16)
            x2shard = P_dram.tile([c.VPC, 128], b16)
            x2t = P_dram.tile([c.NC * c.VPC, 128], b16, addr_space="Shared")

            ps_pool = PS_pool.tile([128, 33], f32, tag="pool")

            # ================= stage 0 =================
            import contextlib
            def rep_loop():
                return tc.For_i(0, reps, 1) if reps > 1 else contextlib.nullcontext()
            NT = c.NPAD // 128
            assert NT % 8 == 0
            for n8 in range(0, NT, 8):
                nb = 8
                xt = P_s0.tile([128, 8 * 128], b16, tag="xt")
                nc.sync.dma_start(out=xt[:, 0:nb * 128],
                                  in_=xT[:, n8 * 128:(n8 + nb) * 128])
                row = P_s0.tile([128, 8 * 256], b16, tag="row")
                for q in range(nb):
                    ps = PS_m.tile([128, 264], f32, tag="m")
                    nc.tensor.matmul(ps[:], lhsT=xt[:, q * 128:(q + 1) * 128],
                                     rhs=Wlr1x_s[:], start=True, stop=True)
                    if q % 2:
                        nc.scalar.copy(out=row[:, q * 256:q * 256 + 256],
                                       in_=ps[:, 0:256])
                    else:
                        nc.vector.tensor_copy(out=row[:, q * 256:q * 256 + 256],
                                              in_=ps[:, 0:256])
                # tab1 row id = blk*1024 + p*8 + t: per-partition rows are
                # contiguous, so the store coalesces to one 4KB descriptor
                # per partition.
                nc.sync.dma_start(
                    out=tab1[n8 * 128:(n8 + nb) * 128, :]
                        .rearrange("(p t) d -> p t d", t=8),
                    in_=row[:, 0:nb * 256].rearrange("p (t d) -> p t d", d=256))
            WCH = 7
            assert c.W % WCH == 0
            for w0 in range(0, c.W, WCH):
                xt = P_s0.tile([128, WCH * 128], b16, tag="xtl")
                nc.sync.dma_start(out=xt[:],
                                  in_=xTloc[:, w0 * 128:(w0 + WCH) * 128])
                for w in range(w0, w0 + WCH):
                    q = w - w0
                    ps = PS_m.tile([128, 264], f32, tag="m")
                    nc.tensor.matmul(ps[:], lhsT=xt[:, q * 128:(q + 1) * 128],
                                     rhs=Wlr1x_s[:], start=True, stop=True)
                    nc.vector.tensor_copy(out=xl1c[:, w * 132:(w + 1) * 132],
                                          in_=ps[:, 0:132])
                    nc.scalar.copy(out=xr1c[:, w * 132:(w + 1) * 132],
                                   in_=ps[:, 132:264])

            # ================= edge layer sweep =================
            gq = [0]  # global gather-call counter for queue rotation
            gsems = [nc.alloc_semaphore(f"gsem{q}") for q in range(4)]
            for _gs in gsems:
                nc.gpsimd.sem_clear(_gs)
            def edge_layer(layer):
                L1 = layer == 1
                MB = 132 if L1 else 33
                FW = 128 if L1 else 32
                NH = 4 if L1 else 1
                GRP = 3 if L1 else (9 if c.T % 9 == 0 else 3)
                NDW = 132 if L1 else 33
                attb = att1b_s if L1 else att2b_s
                WeQ = We1Q_s if L1 else We2Q_s
                res_l = xl1c if L1 else x2c
                res_r = xr1c if L1 else x2c
                rl_w = 132 if L1 else 66
                rr_off = 0 if L1 else 33
                gtab_w = 256 if L1 else 128
                gidxA = idx1A_s if L1 else idx2A_s
                gidxB = idx1B_s if L1 else idx2B_s
                in_apA = tab1[0:c.HALF2 + 512, :] if L1 else x2t[:, :]
                in_apB = tab1[24576:, :] if L1 else x2t[c.HALF2:, :]

                gouts = {}

                def gather_call(stream, k):
                    gidx = gidxA if stream == 0 else gidxB
                    in_ap = in_apA if stream == 0 else in_apB
                    g = P_gat.tile([128, c.CPC * gtab_w], b16, tag=f"g{layer}{stream}")
                    nc.gpsimd.dma_gather(
                        out_ap=g[:].rearrange("p (t d) -> p t d", d=gtab_w),
                        in_ap=in_ap, idxs_ap=gidx[:, k * NW:(k + 1) * NW],
                        num_idxs=c.NIDX, num_idxs_reg=c.NIDX, elem_size=gtab_w,
                        queue_num=gq[0] % 4)
                    gq[0] += 1
                    gouts[(stream, k)] = g

                def xs_slice(stream, s):
                    if 'no_gather' in ablate:
                        return xl1c[:, 0:FW + NH]
                    k, t = divmod(s, c.CPC)
                    return gouts[(stream, k)][:, t * gtab_w: t * gtab_w + FW + NH]

                def issue_upto(w_ahead):
                    if 'no_gather' in ablate:
                        return
                    for stream, TX in ((0, c.TA), (1, c.TB)):
                        last_s = min(c.W, w_ahead + 1) * TX - 1
                        kmax = last_s // c.CPC
                        k0 = 0
                        while (stream, k0) in gouts:
                            k0 += 1
                        for k in range(k0, kmax + 1):
                            if (stream, k) not in gouts:
                                gather_call(stream, k)

                # ---- window finalize: normalize inline, PE tail pipelined ----
                def finalize_a(ps_nd, w):
                    dent = P_win.tile([128, NH], f32, tag="den")
                    nc.vector.tensor_scalar(out=dent[:], in0=ps_nd[:, FW:FW + NH],
                                            scalar1=EPS, scalar2=None, op0=ALU.add)
                    rcpd = P_win.tile([128, NH], f32, tag="rcp")
                    nc.vector.reciprocal(out=rcpd[:], in_=dent[:])
                    if L1:
                        h1r = P_win.tile([128, 128], b16, tag="h1r")
                        for h in range(NH):
                            nc.scalar.activation(
                                out=h1r[:, h * c.CH:(h + 1) * c.CH],
                                in_=ps_nd[:, h * c.CH:(h + 1) * c.CH],
                                func=AF.Relu, scale=rcpd[:, h:h + 1])
                        return h1r
                    else:
                        h2e = P_win.tile([128, 33], b16, tag="h2e")
                        nc.scalar.activation(out=h2e[:, 0:32], in_=ps_nd[:, 0:32],
                                             func=AF.Relu, scale=rcpd[:, 0:1])
                        nc.vector.tensor_copy(out=h2e[:, 32:33], in_=onescol_s[:])
                        ohB = P_win.tile([128, 64], b16, tag="ohB")
                        nc.vector.tensor_tensor(
                            out=ohB[:], in0=iota64_s[:],
                            in1=batchloc_s[:, w:w + 1].to_broadcast([128, 64]),
                            op=ALU.is_equal)
                        return (h2e, ohB)

                def finalize_b(payload, w):
                    if L1:
                        h1r = payload
                        ps_t2 = PS_misc.tile([128, 128], b16, tag="psmisc")
                        nc.tensor.transpose(ps_t2[:], h1r[:], eye_s[:])
                        h1T = P_win.tile([128, 128], b16, tag="h1T")
                        nc.scalar.copy(out=h1T[:], in_=ps_t2[:])
                        ps_x2 = PS_misc.tile([128, 66], f32, tag="psmisc")
                        nc.tensor.matmul(ps_x2[:], lhsT=h1T[:], rhs=Wlr2x_s[:],
                                         start=True, stop=True)
                        nc.vector.tensor_copy(out=x2c[:, w * 66:(w + 1) * 66],
                                              in_=ps_x2[:])
                        sh = P_win.tile([128, 66], b16, tag="sh")
                        nc.scalar.copy(out=sh[:], in_=ps_x2[:])
                        nc.scalar.dma_start(out=x2shard[w * 128:(w + 1) * 128, 0:66],
                                            in_=sh[:])
                    else:
                        h2e, ohB = payload
                        nc.tensor.matmul(ps_pool[0:64, :], lhsT=ohB[:], rhs=h2e[:],
                                         start=(w == 0), stop=(w == c.W - 1))

                pend = [None]

                for w in range(c.W):
                    issue_upto(w + 1)
                    eaT_w = P_ew.tile([16, c.T * 128], b16, tag="eaT")
                    nc.scalar.dma_start(out=eaT_w[:],
                                        in_=eaT[:, w * c.T * 128:(w + 1) * c.T * 128])
                    oh_w = P_ew.tile([128, c.T * 128], b16, tag="oh")
                    nc.sync.dma_start(out=oh_w[:],
                                      in_=ohW[:, w * c.T * 128:(w + 1) * c.T * 128])
                    ohT_w = P_ew.tile([128, c.T * 128], b16, tag="ohT")
                    nc.sync.dma_start(out=ohT_w[:],
                                      in_=ohTW[:, w * c.T * 128:(w + 1) * c.T * 128])

                    ps_nd = PS_nd.tile([128, NDW], f32, tag="nd")
                    first_mm = [True]

                    def nd_mm(lhsT, rhs, stop=False):
                        nc.tensor.matmul(ps_nd[:, 0:rhs.shape[1]], lhsT=lhsT, rhs=rhs,
                                         start=first_mm[0], stop=stop)
                        first_mm[0] = False

                    # ---- self subtile (accumulated into ps_nd via eye) ----
                    ps_s = PS_m.tile([128, MB], f32, tag="m")
                    nc.tensor.matmul(ps_s[:], lhsT=laTh_s[:, w * 128:(w + 1) * 128],
                                     rhs=WeQ[:], start=True, stop=False)
                    nc.tensor.matmul(ps_s[:], lhsT=eye_s[:],
                                     rhs=res_l[:, w * rl_w: w * rl_w + MB],
                                     start=False, stop=False)
                    nc.tensor.matmul(ps_s[:], lhsT=eye_s[:],
                                     rhs=res_r[:, w * rl_w + rr_off: w * rl_w + rr_off + MB],
                                     start=False, stop=True)
                    abs_ = P_win.tile([128, MB], b16, tag="sab")
                    nc.scalar.activation(out=abs_[:], in_=ps_s[:], func=AF.Abs,
                                         scale=(1.0 - NEG) / 2)
                    prs = P_win.tile([128, MB], b16, tag="spr")
                    nc.vector.tensor_tensor(out=prs[:], in0=abs_[:],
                                            in1=attb[:, 0:MB], op=ALU.mult)
                    reds = P_win.tile([128, NH], f32, tag="srd")
                    nc.vector.reduce_sum(
                        out=reds[:],
                        in_=prs[:, 0:FW].rearrange("p (h ch) -> p h ch", ch=c.CH),
                        axis=AX.X)
                    logs = P_win.tile([128, NH], f32, tag="slg")
                    nc.vector.scalar_tensor_tensor(
                        out=logs[:], in0=ps_s[:, FW:FW + NH],
                        scalar=0.5 * (1.0 + NEG), in1=reds[:],
                        op0=ALU.mult, op1=ALU.add)
                    wexps = P_win.tile([128, NH], b16, tag="swx")
                    nc.scalar.activation(out=wexps[:], in_=logs[:], func=AF.Exp)
                    wxws = P_win.tile([128, NDW], b16, tag="sww")
                    nc.vector.tensor_tensor(
                        out=wxws[:, 0:FW].rearrange("p (h ch) -> p h ch", ch=c.CH),
                        in0=res_l[:, w * rl_w: w * rl_w + FW]
                            .rearrange("p (h ch) -> p h ch", ch=c.CH),
                        in1=wexps[:].rearrange("p (h o) -> p h o", o=1)
                            .to_broadcast([128, NH, c.CH]),
                        op=ALU.mult)
                    nc.scalar.copy(out=wxws[:, FW:FW + NH], in_=wexps[:])
                    nd_mm(eye_s[:], wxws[:])
                    for g0 in range(0, c.T, GRP):
                        ps_m = PS_m.tile([128, GRP * MB], f32, tag="m")
                        subs = list(range(g0, g0 + GRP))
                        wxw3 = P_mask.tile([128, GRP * NDW], b16, tag="ww")
                        if L2_DVE_ADD:
                            mbuf = P_grp.tile([128, GRP * MB], b16, tag="mb")
                        for j in subs:
                            jj = j - g0
                            stream = 0 if j < c.TA else 1
                            s_str = (w * c.TA + j) if stream == 0 else (w * c.TB + j - c.TA)
                            k, t = divmod(s_str, c.CPC)
                            if 'no_gather' not in ablate and (stream, k) not in gouts:
                                gather_call(stream, k)
                            if 'no_msgmm' in ablate:
                                continue
                            mb = ps_m[:, jj * MB:(jj + 1) * MB]
                            nc.tensor.matmul(mb, lhsT=eaT_w[:, j * 128:(j + 1) * 128],
                                             rhs=WeQ[:], start=True, stop=False)
                            if not L2_DVE_ADD:
                                nc.tensor.matmul(
                                    mb, lhsT=ohT_w[:, j * 128:(j + 1) * 128],
                                    rhs=res_r[:, w * rl_w + rr_off: w * rl_w + rr_off + MB],
                                    start=False, stop=False)
                                nc.tensor.matmul(mb, lhsT=eye_s[:],
                                                 rhs=xs_slice(stream, s_str),
                                                 start=False, stop=True)
                            else:
                                nc.tensor.matmul(
                                    mb, lhsT=ohT_w[:, j * 128:(j + 1) * 128],
                                    rhs=res_r[:, w * rl_w + rr_off: w * rl_w + rr_off + MB],
                                    start=False, stop=True)
                        if L2_DVE_ADD and 'no_msgmm' not in ablate:
                            aruns = []
                            for j in subs:
                                stream = 0 if j < c.TA else 1
                                s_str = (w * c.TA + j) if stream == 0 else (w * c.TB + j - c.TA)
                                k, t = divmod(s_str, c.CPC)
                                if (aruns and aruns[-1][0] == stream
                                        and aruns[-1][1] == k
                                        and aruns[-1][2] + aruns[-1][3] == t):
                                    aruns[-1][3] += 1
                                else:
                                    aruns.append([stream, k, t, 1, j])
                            for stream, k, t0, nrun, j0 in aruns:
                                jj0 = j0 - g0
                                nc.vector.tensor_tensor(
                                    out=mbuf[:, jj0 * MB:(jj0 + nrun) * MB]
                                        .rearrange("p (t d) -> p t d", d=MB),
                                    in0=ps_m[:, jj0 * MB:(jj0 + nrun) * MB]
                                        .rearrange("p (t d) -> p t d", d=MB),
                                    in1=gouts[(stream, k)]
                                        [:, t0 * gtab_w:(t0 + nrun) * gtab_w]
                                        .rearrange("p (t d) -> p t d", d=gtab_w)
                                        [:, :, 0:MB],
                                    op=ALU.add)
                        # group ACT/DVE chain
                        if 'no_groupchain' in ablate:
                            nc.vector.tensor_copy(out=wxw3[:, 0:GRP * NDW],
                                                  in_=att1b_s[:, 0:1].to_broadcast([128, GRP * NDW]))
                            for j in subs:
                                jj = j - g0
                                nd_mm(oh_w[:, j * 128:(j + 1) * 128],
                                      wxw3[:, jj * NDW:(jj + 1) * NDW],
                                      stop=(j == c.T - 1))
                            continue
                        msrc = mbuf if L2_DVE_ADD else ps_m
                        ab = P_grp.tile([128, GRP * MB], b16, tag="ab")
                        nc.scalar.activation(out=ab[:], in_=msrc[:], func=AF.Abs,
                                             scale=(1.0 - NEG) / 2)
                        prod = P_grp.tile([128, GRP * MB], b16, tag="pr")
                        nc.vector.tensor_tensor(out=prod[:], in0=ab[:],
                                                in1=attb[:, 0:GRP * MB], op=ALU.mult)
                        red = P_grp.tile([128, GRP * NH], f32, tag="rd")
                        nc.vector.reduce_sum(
                            out=red[:].rearrange("p (s h) -> p s h", h=NH),
                            in_=prod[:].rearrange("p (s m) -> p s m", m=MB)[:, :, 0:FW]
                                .rearrange("p s (h ch) -> p s h ch", ch=c.CH),
                            axis=AX.X)
                        logit = P_grp.tile([128, GRP * NH], f32, tag="lg")
                        nc.vector.scalar_tensor_tensor(
                            out=logit[:].rearrange("p (s h) -> p s h", h=NH),
                            in0=msrc[:].rearrange("p (s m) -> p s m", m=MB)
                                [:, :, FW:FW + NH],
                            scalar=0.5 * (1.0 + NEG),
                            in1=red[:].rearrange("p (s h) -> p s h", h=NH),
                            op0=ALU.mult, op1=ALU.add)
                        # per-edge exp weights written straight into the den
                        # cols of the scatter rhs (narrow, strided ACT out)
                        nc.scalar.activation(
                            out=wxw3[:].rearrange("p (s d) -> p s d", d=NDW)
                                [:, :, FW:FW + NH],
                            in_=logit[:].rearrange("p (s h) -> p s h", h=NH),
                            func=AF.Exp)
                        # weighted xs into group rhs buffer, batched per
                        # contiguous run within one gather tile
                        runs = []
                        for j in subs:
                            stream = 0 if j < c.TA else 1
                            s_str = (w * c.TA + j) if stream == 0 else (w * c.TB + j - c.TA)
                            k, t = divmod(s_str, c.CPC)
                            if ('no_gather' not in ablate and runs
                                    and runs[-1][0] == stream and runs[-1][1] == k
                                    and runs[-1][2] + runs[-1][3] == t):
                                runs[-1][3] += 1
                            else:
                                runs.append([stream, k, t, 1, j])
                        for stream, k, t0, nrun, j0 in runs:
                            jj0 = j0 - g0
                            if 'no_gather' in ablate:
                                gsl = xl1c[:, 0:nrun * gtab_w]
                            else:
                                gsl = gouts[(stream, k)][:, t0 * gtab_w:
                                                         (t0 + nrun) * gtab_w]
                            nc.vector.tensor_tensor(
                                out=wxw3[:, jj0 * NDW: (jj0 + nrun) * NDW]
                                    .rearrange("p (t d) -> p t d", d=NDW)[:, :, 0:FW]
                                    .rearrange("p t (h ch) -> p t h ch", ch=c.CH),
                                in0=gsl
                                    .rearrange("p (t d) -> p t d", d=gtab_w)[:, :, 0:FW]
                                    .rearrange("p t (h ch) -> p t h ch", ch=c.CH),
                                in1=wxw3[:, jj0 * NDW: (jj0 + nrun) * NDW]
                                    .rearrange("p (t d) -> p t d", d=NDW)
                                    [:, :, FW:FW + NH]
                                    .rearrange("p t (h o) -> p t h o", o=1)
                                    .to_broadcast([128, nrun, NH, c.CH]),
                                op=ALU.mult)
                        for j in subs:
                            jj = j - g0
                            if 'no_agg' in ablate and j != 0:
                                continue
                            nd_mm(oh_w[:, j * 128:(j + 1) * 128],
                                  wxw3[:, jj * NDW:(jj + 1) * NDW],
                                  stop=(j == c.T - 1))

                    payload = finalize_a(ps_nd, w)
                    if pend[0] is not None:
                        finalize_b(*pend[0])
                    pend[0] = (payload, w)
                finalize_b(*pend[0])

            with rep_loop():
                edge_layer(1)

            if skip_collective:
                nc.sync.dma_start(out=x2t[0:c.VPC, :], in_=x2shard[:, :])
            else:
                nc.gpsimd.collective_compute(
                    "AllGather", ALU.bypass,
                    ins=[x2shard[:].opt()], outs=[x2t[:].opt()],
                    replica_groups=[list(range(c.NC))])

            with rep_loop():
                edge_layer(2)

            pout = P_win.tile([64, 33], f32, tag="pout")
            nc.vector.tensor_copy(out=pout[:], in_=ps_pool[0:64, :])
            nc.sync.dma_start(out=partial[:, :], in_=pout[:])

    nc.compile()
    return nc


# ======================= host side =======================

def host_prep(inputs, c: Cfg):
    x = np.asarray(inputs['x'], np.float32)
    ei = np.asarray(inputs['edge_index'])
    ea = np.asarray(inputs['edge_attr'], np.float32)
    batch = np.asarray(inputs['batch'])
    src, dst = np.asarray(ei[0], np.int64), np.asarray(ei[1], np.int64)
    Wl1 = np.asarray(inputs['Wl1'], np.float32); Wr1 = np.asarray(inputs['Wr1'], np.float32)
    We1 = np.asarray(inputs['We1'], np.float32); att1 = np.asarray(inputs['att1'], np.float32)
    Wl2 = np.asarray(inputs['Wl2'], np.float32); Wr2 = np.asarray(inputs['Wr2'], np.float32)
    We2 = np.asarray(inputs['We2'], np.float32); att2 = np.asarray(inputs['att2'], np.float32)
    assert float(np.abs(np.asarray(inputs['b1'])).max()) == 0.0
    assert float(np.abs(np.asarray(inputs['b2'])).max()) == 0.0

    HEADS, CH, HC = c.HEADS, c.CH, c.HC
    att_bd = np.zeros((HC, HEADS), np.float32)
    for h in range(HEADS):
        att_bd[h * CH:(h + 1) * CH, h] = att1[h]
    a2 = att2.reshape(c.HID, 1)

    # self-loop attr: mean incoming edge_attr per node (0 for isolated nodes)
    N = c.N
    order = np.argsort(dst, kind='stable')
    ds = dst[order]; eas = ea[order]
    bounds = np.searchsorted(ds, np.arange(N))
    bsafe = np.minimum(bounds, len(ds) - 1)
    cnt = np.bincount(dst, minlength=N).astype(np.float32)
    sums = np.add.reduceat(eas, bsafe, axis=0)
    sums[cnt == 0] = 0.0
    loop_attr = sums / np.maximum(cnt, 1.0)[:, None]

    xTg = np.zeros((128, c.NPAD), bf16)
    xTg[:, :c.N] = x.T.astype(bf16)
    Wlr1x = np.concatenate([Wl1, Wl1 @ att_bd, Wr1, Wr1 @ att_bd], 1).astype(bf16)
    We1Q = np.concatenate([We1, We1 @ att_bd], 1).astype(bf16)
    We2Q = np.concatenate([We2, We2 @ a2], 1).astype(bf16)
    Wlr2x = np.concatenate([Wl2, Wl2 @ a2, Wr2, Wr2 @ a2], 1).astype(bf16)
    att1b = np.zeros((128, 396), bf16)
    for s in range(3):
        att1b[:, s * 132:s * 132 + 128] = att1.reshape(-1).astype(bf16)[None, :]
    att2b = np.zeros((128, 297), bf16)
    for s in range(9):
        att2b[:, s * 33:s * 33 + 32] = att2.reshape(-1).astype(bf16)[None, :]
    eye_ = np.eye(128, dtype=np.float32).astype(bf16)
    shared = dict(xT=xTg, Wlr1x=Wlr1x, We1Q=We1Q, We2Q=We2Q, Wlr2x=Wlr2x,
                  att1b=att1b, att2b=att2b,
                  eye_bf=eye_,
                  onescol=np.ones((128, 1), bf16),
                  iota64=np.tile(np.arange(64, dtype=np.float32)[None, :], (128, 1)))
    eye129 = np.vstack([np.eye(128, dtype=np.float32).astype(bf16),
                        np.zeros((1, 128), bf16)])

    in_maps = []
    for core in range(c.NC):
        c0 = core * c.VPCr
        m = (dst >= c0) & (dst < c0 + c.VPCr)
        s_c = src[m]; d_c = dst[m] - c0; e_c = ea[m]
        w_c = d_c >> 7
        half_c = (s_c >= c.HALF).astype(np.int64)

        E_slots = c.W * c.T * 128
        slot_src = np.zeros(E_slots, np.int64)
        slot_dl = np.full(E_slots, -1.0, np.float32)
        slot_ea = np.zeros((E_slots, 16), np.float32)
        order = np.lexsort((half_c, w_c))
        s_o, d_o, w_o, h_o, e_o = (s_c[order], d_c[order], w_c[order],
                                   half_c[order], e_c[order])
        nE = len(s_o)
        # group boundaries: edges sorted by (w, half)
        bounds = np.searchsorted(w_o * 2 + h_o, np.arange(c.W * 2 + 1))
        for w in range(c.W):
            for hh in (0, 1):
                lo, hi = bounds[w * 2 + hh], bounds[w * 2 + hh + 1]
                n = hi - lo
                lim = (c.TA if hh == 0 else c.TB) * 128
                assert n <= lim, (core, w, hh, n, lim)
                base = w * c.T * 128 + (0 if hh == 0 else c.TA * 128)
                slot_src[base:base + n] = s_o[lo:hi]
                slot_dl[base:base + n] = (d_o[lo:hi] - w * 128).astype(np.float32)
                slot_ea[base:base + n] = e_o[lo:hi]

        eaT_a = np.ascontiguousarray(slot_ea.T).astype(bf16)

        # one-hot scatter tables (host-precomputed)
        dl3i = slot_dl.reshape(c.W * c.T, 128).astype(np.int64)
        dl3i[dl3i < 0] = 128
        A = eye129[dl3i]                       # [S, p=edge, n] bf16
        ohW_a = np.ascontiguousarray(A.transpose(1, 0, 2)).reshape(128, -1)
        ohTW_a = np.ascontiguousarray(A.transpose(2, 0, 1)).reshape(128, -1)

        # self-loop mean-attr table, transposed per window
        la_core = np.zeros((c.VPC, 16), np.float32)
        la_core[:c.VPCr] = loop_attr[c0:c0 + c.VPCr]
        laTh_a = np.ascontiguousarray(la_core.T).astype(bf16)

        def build_idx(vals, ncall):
            out = np.zeros((128, ncall * (c.NIDX // 16)), np.int16)
            v = vals.reshape(ncall, c.NIDX)
            ii = np.arange(c.NIDX)
            for k in range(ncall):
                blk = np.zeros((16, c.NIDX // 16), np.int16)
                blk[ii % 16, ii // 16] = v[k].astype(np.int16)
                out[:, k * (c.NIDX // 16):(k + 1) * (c.NIDX // 16)] = np.tile(blk, (8, 1))
            return out

        slots3 = slot_src.reshape(c.W, c.T, 128)
        dl3 = slot_dl.reshape(c.W, c.T, 128)
        A_src = slots3[:, :c.TA, :].reshape(-1)
        B_src = slots3[:, c.TA:, :].reshape(-1)
        A_pad = dl3[:, :c.TA, :].reshape(-1) < 0
        B_pad = dl3[:, c.TA:, :].reshape(-1) < 0
        def r1(v):
            # tab1 storage row for node v (see stage-0 store coalescing)
            return (v >> 10) * 1024 + (v & 127) * 8 + ((v >> 7) & 7)
        i1A = np.where(A_pad, 0, r1(A_src))
        i1B = np.where(B_pad, 0, r1(B_src) - 24576)
        i2A = np.where(A_pad, 0, (A_src // c.VPCr) * c.VPC + (A_src % c.VPCr))
        i2B = np.where(B_pad, 0,
                       (B_src // c.VPCr) * c.VPC + (B_src % c.VPCr) - c.HALF2)
        for a in (i1A, i1B, i2A, i2B):
            assert a.min() >= 0 and a.max() < 32768

        xTloc_a = np.zeros((128, c.VPC), bf16)
        nreal = c.VPCr
        xTloc_a[:, :nreal] = x[c0:c0 + nreal].T.astype(bf16)

        blfull = np.full(c.VPC, -1.0, np.float32)
        blfull[:nreal] = np.asarray(batch[c0:c0 + nreal], np.float32)
        bl = blfull.reshape(c.W, 128).T.copy()

        im = dict(shared)
        im.update(xTloc=xTloc_a, eaT=eaT_a, ohW=ohW_a, ohTW=ohTW_a,
                  laTh=laTh_a,
                  idx1A=build_idx(i1A, c.NCALLA), idx1B=build_idx(i1B, c.NCALLB),
                  idx2A=build_idx(i2A, c.NCALLA), idx2B=build_idx(i2B, c.NCALLB),
                  batchloc=bl)
        in_maps.append(im)

    ctx = dict(Wc=np.asarray(inputs['Wc'], np.float32),
               bc=np.asarray(inputs['bc'], np.float32), G=c.G)
    return in_maps, ctx


def host_finalize(partials, ctx):
    tot = np.zeros((64, 33), np.float64)
    for p in partials:
        tot += np.asarray(p, np.float64)
    G = ctx['G']
    pooled = tot[:G, 0:32] / np.maximum(tot[:G, 32:33], 1.0)
    out = pooled.astype(np.float32) @ ctx['Wc'] + ctx['bc']
    return out.astype(np.float32)


# ======================= kernel entry =======================
_CACHE = {}


def _get_program(cfg_key, c):
    if cfg_key not in _CACHE:
        _CACHE[cfg_key] = build_program(c)
    return _CACHE[cfg_key]


def kernel(**inputs):
    """Full-input GATv2 kernel on 8 TRN2 NeuronCores. Returns [64, 2] f32."""
    from concourse import bass_utils

    ei = np.asarray(inputs['edge_index'])
    src = np.asarray(ei[0], np.int64)
    dst = np.asarray(ei[1], np.int64)
    N = int(np.asarray(inputs['x']).shape[0])
    NC = 8
    assert N % NC == 0, N
    VPCr = N // NC
    W = (VPCr + 127) // 128
    HALF = N // 2
    maxTA = maxTB = 1
    for core in range(NC):
        m = (dst >= core * VPCr) & (dst < (core + 1) * VPCr)
        w = (dst[m] - core * VPCr) >> 7
        hh = src[m] >= HALF
        cA = np.bincount(w[~hh], minlength=W)
        cB = np.bincount(w[hh], minlength=W)
        maxTA = max(maxTA, int(((cA + 127) // 128).max()))
        maxTB = max(maxTB, int(((cB + 127) // 128).max()))
    while (W * maxTA) % 7:
        maxTA += 1
    while (W * maxTB) % 7:
        maxTB += 1
    while (maxTA + maxTB) % 3:
        maxTB += 1

    c = Cfg(NC=NC, VPCr=VPCr, TA=maxTA, TB=maxTB, G=64)
    in_maps, ctx = host_prep(inputs, c)
    nc = _get_program((NC, VPCr, maxTA, maxTB), c)
    res = bass_utils.run_bass_kernel_spmd(nc, in_maps, core_ids=list(range(NC)))
    partials = [res.results[i]["partial"] for i in range(NC)]
    return host_finalize(partials, ctx)



# revision 38
# speedup vs baseline: 1.0852x; 1.0852x over previous
"""GATv2 (2-layer, 4+1 heads) TRN2 bass kernel, 8-core SPMD.

Accepts FULL inputs as produced by reference.setup_inputs() and returns the
FULL [64, 2] output. Internally: edges are partitioned by destination core
(6250 nodes/core) and destination 128-node window, padded to a static
(TA, TB) tile schedule; per-edge messages are assembled in PSUM (edge-attr
and one-hot-expanded x_r matmuls; gathered x_l joins via a third matmul in
layer 1 and a batched DVE add in layer 2); leaky-relu is evaluated as
0.6*linear + 0.4*|m| with the linear logit part riding extra matmul columns;
softmax runs max-free (logits are bounded); aggregation uses onehot-matmul
scatter into per-window PSUM accumulators, with the gather-free self-loop
subtile computed first so it stays off the window critical path, and the
1/den normalization + relu folded into per-head scalar-engine activations.
The one-hot scatter tables (oh / ohT) and the self-loop mean edge-attr table
are precomputed on the host and streamed from HBM; stage-0 stores coalesce
to 4KB descriptors and the A-stream gathers depend only on the A half of
tab1 so they start mid-stage-0. Layer-2 tables are exchanged with an
on-device AllGather. Per-core pooled partials [64, 33] are combined on the
host with the final [32x2] classifier.
"""
import sys
for _p in ('/opt/trn_rl_repo', '/root/.axon_site/_ro/trn_rl_repo'):
    if _p not in sys.path:
        sys.path.insert(0, _p)

import numpy as np
import ml_dtypes

import concourse.bass as bass
import concourse.bacc as bacc
import concourse.mybir as mybir
import concourse.tile as tile

bf16 = ml_dtypes.bfloat16
AF = mybir.ActivationFunctionType
ALU = mybir.AluOpType
AX = mybir.AxisListType
DT = mybir.dt
NEG = 0.2
EPS = 1e-16
L2_DVE_ADD = True
L1_DVE_ADD = False


class Cfg:
    def __init__(self, NC=8, VPCr=6250, TA=9, TB=9, G=64):
        self.NC = NC
        self.VPCr = VPCr
        self.N = NC * VPCr
        self.W = (VPCr + 127) // 128
        self.VPC = self.W * 128
        self.NPAD = NC * self.VPC
        self.TA, self.TB = TA, TB
        self.T = TA + TB
        self.G = G
        self.HALF = self.N // 2
        self.HALF2 = (NC // 2) * self.VPC
        self.HC = 128
        self.HEADS = 4
        self.CH = 32
        self.HID = 32
        self.SA = self.W * TA
        self.SB = self.W * TB
        self.CPC = max(d for d in (7, 3, 1)
                       if self.SA % d == 0 and self.SB % d == 0)
        self.NCALLA = self.SA // self.CPC
        self.NCALLB = self.SB // self.CPC
        self.NIDX = self.CPC * 128
        assert self.T % 3 == 0
        assert NC % 2 == 0 and VPCr % 2 == 0


def build_program(c: Cfg, debug=False, reps=1, skip_collective=False, ablate=()):
    nc = bacc.Bacc("TRN2", target_bir_lowering=False, debug=debug,
                   num_swdge_queues=4)
    f32, b16, i16 = DT.float32, DT.bfloat16, DT.int16

    def inp(name, shape, dt=f32):
        return nc.dram_tensor(name, shape, dt, kind="ExternalInput")

    xT = inp("xT", [128, c.NPAD], b16)
    xTloc = inp("xTloc", [128, c.VPC], b16)
    Wlr1x = inp("Wlr1x", [128, 264], b16)
    We1Q = inp("We1Q", [16, 132], b16)
    We2Q = inp("We2Q", [16, 33], b16)
    Wlr2x = inp("Wlr2x", [128, 66], b16)
    att1b = inp("att1b", [128, 396], b16)
    att2b = inp("att2b", [128, 297], b16)
    eye_bf = inp("eye_bf", [128, 128], b16)
    onescol = inp("onescol", [128, 1], b16)
    iota64 = inp("iota64", [128, 64], f32)
    eaT = inp("eaT", [16, c.W * c.T * 128], b16)
    ohW = inp("ohW", [128, c.W * c.T * 128], b16)
    ohTW = inp("ohTW", [128, c.W * c.T * 128], b16)
    laTh = inp("laTh", [16, c.W * 128], b16)
    NW = c.NIDX // 16
    idx1A = inp("idx1A", [128, c.NCALLA * NW], i16)
    idx1B = inp("idx1B", [128, c.NCALLB * NW], i16)
    idx2A = inp("idx2A", [128, c.NCALLA * NW], i16)
    idx2B = inp("idx2B", [128, c.NCALLB * NW], i16)
    batchloc = inp("batchloc", [128, c.W], f32)

    partial = nc.dram_tensor("partial", [64, 33], f32, kind="ExternalOutput")

    with tile.TileContext(nc) as tc:
        with (
            tc.tile_pool(name="const", bufs=1) as P_const,
            tc.tile_pool(name="res", bufs=1) as P_res,
            tc.tile_pool(name="s0", bufs=2) as P_s0,
            tc.tile_pool(name="gat", bufs=6) as P_gat,
            tc.tile_pool(name="ew", bufs=3) as P_ew,
            tc.tile_pool(name="mask", bufs=3) as P_mask,
            tc.tile_pool(name="grp", bufs=3) as P_grp,
            tc.tile_pool(name="win", bufs=2) as P_win,
            tc.tile_pool(name="pm", bufs=3, space="PSUM") as PS_m,
            tc.tile_pool(name="pnd", bufs=2, space="PSUM") as PS_nd,
            tc.tile_pool(name="pmisc", bufs=2, space="PSUM") as PS_misc,
            tc.tile_pool(name="ppool", bufs=1, space="PSUM") as PS_pool,
            tc.tile_pool(name="dram", bufs=1, space="DRAM") as P_dram,
        ):
            def load_const(t, shape, dt):
                s = P_const.tile(shape, dt, tag=t.name)
                nc.sync.dma_start(out=s[:], in_=t[:, :])
                return s

            Wlr1x_s = load_const(Wlr1x, [128, 264], b16)
            We1Q_s = load_const(We1Q, [16, 132], b16)
            We2Q_s = load_const(We2Q, [16, 33], b16)
            Wlr2x_s = load_const(Wlr2x, [128, 66], b16)
            att1b_s = load_const(att1b, [128, 396], b16)
            att2b_s = load_const(att2b, [128, 297], b16)
            eye_s = load_const(eye_bf, [128, 128], b16)
            onescol_s = load_const(onescol, [128, 1], b16)
            iota64_s = load_const(iota64, [128, 64], f32)
            laTh_s = load_const(laTh, [16, c.W * 128], b16)
            idx1A_s = load_const(idx1A, [128, c.NCALLA * NW], i16)
            idx1B_s = load_const(idx1B, [128, c.NCALLB * NW], i16)
            idx2A_s = load_const(idx2A, [128, c.NCALLA * NW], i16)
            idx2B_s = load_const(idx2B, [128, c.NCALLB * NW], i16)
            batchloc_s = load_const(batchloc, [128, c.W], f32)

            xl1c = P_res.tile([128, c.W * 132], b16)
            xr1c = P_res.tile([128, c.W * 132], b16)
            x2c = P_res.tile([128, c.W * 66], b16)

            tab1 = P_dram.tile([c.NPAD, 256], b16)
            x2shard = P_dram.tile([c.VPC, 128], b16)
            x2t = P_dram.tile([c.NC * c.VPC, 128], b16, addr_space="Shared")

            ps_pool = PS_pool.tile([128, 33], f32, tag="pool")

            # ================= stage 0 =================
            import contextlib
            def rep_loop():
                return tc.For_i(0, reps, 1) if reps > 1 else contextlib.nullcontext()
            NT = c.NPAD // 128
            assert NT % 8 == 0
            for n8 in range(0, NT, 8):
                nb = 8
                xt = P_s0.tile([128, 8 * 128], b16, tag="xt")
                nc.sync.dma_start(out=xt[:, 0:nb * 128],
                                  in_=xT[:, n8 * 128:(n8 + nb) * 128])
                row = P_s0.tile([128, 8 * 256], b16, tag="row")
                for q in range(nb):
                    ps = PS_m.tile([128, 264], f32, tag="m")
                    nc.tensor.matmul(ps[:], lhsT=xt[:, q * 128:(q + 1) * 128],
                                     rhs=Wlr1x_s[:], start=True, stop=True)
                    if q % 2:
                        nc.scalar.copy(out=row[:, q * 256:q * 256 + 256],
                                       in_=ps[:, 0:256])
                    else:
                        nc.vector.tensor_copy(out=row[:, q * 256:q * 256 + 256],
                                              in_=ps[:, 0:256])
                # tab1 row id = blk*1024 + p*8 + t: per-partition rows are
                # contiguous, so the store coalesces to one 4KB descriptor
                # per partition.
                nc.sync.dma_start(
                    out=tab1[n8 * 128:(n8 + nb) * 128, :]
                        .rearrange("(p t) d -> p t d", t=8),
                    in_=row[:, 0:nb * 256].rearrange("p (t d) -> p t d", d=256))
            WCH = 7
            assert c.W % WCH == 0
            for w0 in range(0, c.W, WCH):
                xt = P_s0.tile([128, WCH * 128], b16, tag="xtl")
                nc.sync.dma_start(out=xt[:],
                                  in_=xTloc[:, w0 * 128:(w0 + WCH) * 128])
                for w in range(w0, w0 + WCH):
                    q = w - w0
                    ps = PS_m.tile([128, 264], f32, tag="m")
                    nc.tensor.matmul(ps[:], lhsT=xt[:, q * 128:(q + 1) * 128],
                                     rhs=Wlr1x_s[:], start=True, stop=True)
                    nc.vector.tensor_copy(out=xl1c[:, w * 132:(w + 1) * 132],
                                          in_=ps[:, 0:132])
                    nc.scalar.copy(out=xr1c[:, w * 132:(w + 1) * 132],
                                   in_=ps[:, 132:264])

            # ================= edge layer sweep =================
            gq = [0]  # global gather-call counter for queue rotation
            gsems = [nc.alloc_semaphore(f"gsem{q}") for q in range(4)]
            for _gs in gsems:
                nc.gpsimd.sem_clear(_gs)
            def edge_layer(layer):
                L1 = layer == 1
                MB = 132 if L1 else 33
                FW = 128 if L1 else 32
                NH = 4 if L1 else 1
                GRP = 3 if L1 else (9 if c.T % 9 == 0 else 3)
                NDW = 132 if L1 else 33
                attb = att1b_s if L1 else att2b_s
                WeQ = We1Q_s if L1 else We2Q_s
                res_l = xl1c if L1 else x2c
                res_r = xr1c if L1 else x2c
                rl_w = 132 if L1 else 66
                rr_off = 0 if L1 else 33
                gtab_w = 256 if L1 else 128
                gidxA = idx1A_s if L1 else idx2A_s
                gidxB = idx1B_s if L1 else idx2B_s
                in_apA = tab1[0:c.HALF2 + 512, :] if L1 else x2t[:, :]
                in_apB = tab1[24576:, :] if L1 else x2t[c.HALF2:, :]

                gouts = {}

                def gather_call(stream, k):
                    gidx = gidxA if stream == 0 else gidxB
                    in_ap = in_apA if stream == 0 else in_apB
                    g = P_gat.tile([128, c.CPC * gtab_w], b16, tag=f"g{layer}{stream}")
                    nc.gpsimd.dma_gather(
                        out_ap=g[:].rearrange("p (t d) -> p t d", d=gtab_w),
                        in_ap=in_ap, idxs_ap=gidx[:, k * NW:(k + 1) * NW],
                        num_idxs=c.NIDX, num_idxs_reg=c.NIDX, elem_size=gtab_w,
                        queue_num=gq[0] % 4)
                    gq[0] += 1
                    gouts[(stream, k)] = g

                def xs_slice(stream, s):
                    if 'no_gather' in ablate:
                        return xl1c[:, 0:FW + NH]
                    k, t = divmod(s, c.CPC)
                    return gouts[(stream, k)][:, t * gtab_w: t * gtab_w + FW + NH]

                def issue_upto(w_ahead):
                    if 'no_gather' in ablate:
                        return
                    for stream, TX in ((0, c.TA), (1, c.TB)):
                        last_s = min(c.W, w_ahead + 1) * TX - 1
                        kmax = last_s // c.CPC
                        k0 = 0
                        while (stream, k0) in gouts:
                            k0 += 1
                        for k in range(k0, kmax + 1):
                            if (stream, k) not in gouts:
                                gather_call(stream, k)

                # ---- window finalize: normalize inline, PE tail pipelined ----
                def finalize_a(ps_nd, w):
                    dent = P_win.tile([128, NH], f32, tag="den")
                    nc.vector.tensor_scalar(out=dent[:], in0=ps_nd[:, FW:FW + NH],
                                            scalar1=EPS, scalar2=None, op0=ALU.add)
                    rcpd = P_win.tile([128, NH], f32, tag="rcp")
                    nc.vector.reciprocal(out=rcpd[:], in_=dent[:])
                    if L1:
                        h1r = P_win.tile([128, 128], b16, tag="h1r")
                        for h in range(NH):
                            nc.scalar.activation(
                                out=h1r[:, h * c.CH:(h + 1) * c.CH],
                                in_=ps_nd[:, h * c.CH:(h + 1) * c.CH],
                                func=AF.Relu, scale=rcpd[:, h:h + 1])
                        return h1r
                    else:
                        h2e = P_win.tile([128, 33], b16, tag="h2e")
                        nc.scalar.activation(out=h2e[:, 0:32], in_=ps_nd[:, 0:32],
                                             func=AF.Relu, scale=rcpd[:, 0:1])
                        nc.vector.tensor_copy(out=h2e[:, 32:33], in_=onescol_s[:])
                        ohB = P_win.tile([128, 64], b16, tag="ohB")
                        nc.vector.tensor_tensor(
                            out=ohB[:], in0=iota64_s[:],
                            in1=batchloc_s[:, w:w + 1].to_broadcast([128, 64]),
                            op=ALU.is_equal)
                        return (h2e, ohB)

                def finalize_b(payload, w):
                    if L1:
                        h1r = payload
                        ps_t2 = PS_misc.tile([128, 128], b16, tag="psmisc")
                        nc.tensor.transpose(ps_t2[:], h1r[:], eye_s[:])
                        h1T = P_win.tile([128, 128], b16, tag="h1T")
                        nc.scalar.copy(out=h1T[:], in_=ps_t2[:])
                        ps_x2 = PS_misc.tile([128, 66], f32, tag="psmisc")
                        nc.tensor.matmul(ps_x2[:], lhsT=h1T[:], rhs=Wlr2x_s[:],
                                         start=True, stop=True)
                        nc.vector.tensor_copy(out=x2c[:, w * 66:(w + 1) * 66],
                                              in_=ps_x2[:])
                        sh = P_win.tile([128, 66], b16, tag="sh")
                        nc.scalar.copy(out=sh[:], in_=ps_x2[:])
                        nc.scalar.dma_start(out=x2shard[w * 128:(w + 1) * 128, 0:66],
                                            in_=sh[:])
                    else:
                        h2e, ohB = payload
                        nc.tensor.matmul(ps_pool[0:64, :], lhsT=ohB[:], rhs=h2e[:],
                                         start=(w == 0), stop=(w == c.W - 1))

                pend = [None]

                for w in range(c.W):
                    issue_upto(w + 1)
                    eaT_w = P_ew.tile([16, c.T * 128], b16, tag="eaT")
                    nc.scalar.dma_start(out=eaT_w[:],
                                        in_=eaT[:, w * c.T * 128:(w + 1) * c.T * 128])
                    oh_w = P_ew.tile([128, c.T * 128], b16, tag="oh")
                    nc.sync.dma_start(out=oh_w[:],
                                      in_=ohW[:, w * c.T * 128:(w + 1) * c.T * 128])
                    ohT_w = P_ew.tile([128, c.T * 128], b16, tag="ohT")
                    nc.sync.dma_start(out=ohT_w[:],
                                      in_=ohTW[:, w * c.T * 128:(w + 1) * c.T * 128])

                    ps_nd = PS_nd.tile([128, NDW], f32, tag="nd")
                    first_mm = [True]

                    def nd_mm(lhsT, rhs, stop=False):
                        nc.tensor.matmul(ps_nd[:, 0:rhs.shape[1]], lhsT=lhsT, rhs=rhs,
                                         start=first_mm[0], stop=stop)
                        first_mm[0] = False

                    # ---- self subtile (accumulated into ps_nd via eye) ----
                    ps_s = PS_m.tile([128, MB], f32, tag="m")
                    nc.tensor.matmul(ps_s[:], lhsT=laTh_s[:, w * 128:(w + 1) * 128],
                                     rhs=WeQ[:], start=True, stop=False)
                    nc.tensor.matmul(ps_s[:], lhsT=eye_s[:],
                                     rhs=res_l[:, w * rl_w: w * rl_w + MB],
                                     start=False, stop=False)
                    nc.tensor.matmul(ps_s[:], lhsT=eye_s[:],
                                     rhs=res_r[:, w * rl_w + rr_off: w * rl_w + rr_off + MB],
                                     start=False, stop=True)
                    abs_ = P_win.tile([128, MB], b16, tag="sab")
                    nc.scalar.activation(out=abs_[:], in_=ps_s[:], func=AF.Abs,
                                         scale=(1.0 - NEG) / 2)
                    prs = P_win.tile([128, MB], b16, tag="spr")
                    nc.vector.tensor_tensor(out=prs[:], in0=abs_[:],
                                            in1=attb[:, 0:MB], op=ALU.mult)
                    reds = P_win.tile([128, NH], f32, tag="srd")
                    nc.vector.reduce_sum(
                        out=reds[:],
                        in_=prs[:, 0:FW].rearrange("p (h ch) -> p h ch", ch=c.CH),
                        axis=AX.X)
                    logs = P_win.tile([128, NH], f32, tag="slg")
                    nc.vector.scalar_tensor_tensor(
                        out=logs[:], in0=ps_s[:, FW:FW + NH],
                        scalar=0.5 * (1.0 + NEG), in1=reds[:],
                        op0=ALU.mult, op1=ALU.add)
                    wexps = P_win.tile([128, NH], b16, tag="swx")
                    nc.scalar.activation(out=wexps[:], in_=logs[:], func=AF.Exp)
                    wxws = P_win.tile([128, NDW], b16, tag="sww")
                    nc.vector.tensor_tensor(
                        out=wxws[:, 0:FW].rearrange("p (h ch) -> p h ch", ch=c.CH),
                        in0=res_l[:, w * rl_w: w * rl_w + FW]
                            .rearrange("p (h ch) -> p h ch", ch=c.CH),
                        in1=wexps[:].rearrange("p (h o) -> p h o", o=1)
                            .to_broadcast([128, NH, c.CH]),
                        op=ALU.mult)
                    nc.scalar.copy(out=wxws[:, FW:FW + NH], in_=wexps[:])
                    nd_mm(eye_s[:], wxws[:])
                    for g0 in range(0, c.T, GRP):
                        ps_m = PS_m.tile([128, GRP * MB], f32, tag="m")
                        subs = list(range(g0, g0 + GRP))
                        wxw3 = P_mask.tile([128, GRP * NDW], b16, tag="ww")
                        dve_add = L1_DVE_ADD if L1 else L2_DVE_ADD
                        if dve_add:
                            mbuf = P_grp.tile([128, GRP * MB], b16, tag="mb")
                        for j in subs:
                            jj = j - g0
                            stream = 0 if j < c.TA else 1
                            s_str = (w * c.TA + j) if stream == 0 else (w * c.TB + j - c.TA)
                            k, t = divmod(s_str, c.CPC)
                            if 'no_gather' not in ablate and (stream, k) not in gouts:
                                gather_call(stream, k)
                            if 'no_msgmm' in ablate:
                                continue
                            mb = ps_m[:, jj * MB:(jj + 1) * MB]
                            nc.tensor.matmul(mb, lhsT=eaT_w[:, j * 128:(j + 1) * 128],
                                             rhs=WeQ[:], start=True, stop=False)
                            if not dve_add:
                                nc.tensor.matmul(
                                    mb, lhsT=ohT_w[:, j * 128:(j + 1) * 128],
                                    rhs=res_r[:, w * rl_w + rr_off: w * rl_w + rr_off + MB],
                                    start=False, stop=False)
                                nc.tensor.matmul(mb, lhsT=eye_s[:],
                                                 rhs=xs_slice(stream, s_str),
                                                 start=False, stop=True)
                            else:
                                nc.tensor.matmul(
                                    mb, lhsT=ohT_w[:, j * 128:(j + 1) * 128],
                                    rhs=res_r[:, w * rl_w + rr_off: w * rl_w + rr_off + MB],
                                    start=False, stop=True)
                        if dve_add and 'no_msgmm' not in ablate:
                            aruns = []
                            for j in subs:
                                stream = 0 if j < c.TA else 1
                                s_str = (w * c.TA + j) if stream == 0 else (w * c.TB + j - c.TA)
                                k, t = divmod(s_str, c.CPC)
                                if (aruns and aruns[-1][0] == stream
                                        and aruns[-1][1] == k
                                        and aruns[-1][2] + aruns[-1][3] == t):
                                    aruns[-1][3] += 1
                                else:
                                    aruns.append([stream, k, t, 1, j])
                            for stream, k, t0, nrun, j0 in aruns:
                                jj0 = j0 - g0
                                nc.vector.tensor_tensor(
                                    out=mbuf[:, jj0 * MB:(jj0 + nrun) * MB]
                                        .rearrange("p (t d) -> p t d", d=MB),
                                    in0=ps_m[:, jj0 * MB:(jj0 + nrun) * MB]
                                        .rearrange("p (t d) -> p t d", d=MB),
                                    in1=gouts[(stream, k)]
                                        [:, t0 * gtab_w:(t0 + nrun) * gtab_w]
                                        .rearrange("p (t d) -> p t d", d=gtab_w)
                                        [:, :, 0:MB],
                                    op=ALU.add)
                        # group ACT/DVE chain
                        if 'no_groupchain' in ablate:
                            nc.vector.tensor_copy(out=wxw3[:, 0:GRP * NDW],
                                                  in_=att1b_s[:, 0:1].to_broadcast([128, GRP * NDW]))
                            for j in subs:
                                jj = j - g0
                                nd_mm(oh_w[:, j * 128:(j + 1) * 128],
                                      wxw3[:, jj * NDW:(jj + 1) * NDW],
                                      stop=(j == c.T - 1))
                            continue
                        msrc = mbuf if dve_add else ps_m
                        ab = P_grp.tile([128, GRP * MB], b16, tag="ab")
                        nc.scalar.activation(out=ab[:], in_=msrc[:], func=AF.Abs,
                                             scale=(1.0 - NEG) / 2)
                        prod = P_grp.tile([128, GRP * MB], b16, tag="pr")
                        nc.vector.tensor_tensor(out=prod[:], in0=ab[:],
                                                in1=attb[:, 0:GRP * MB], op=ALU.mult)
                        red = P_grp.tile([128, GRP * NH], f32, tag="rd")
                        nc.vector.reduce_sum(
                            out=red[:].rearrange("p (s h) -> p s h", h=NH),
                            in_=prod[:].rearrange("p (s m) -> p s m", m=MB)[:, :, 0:FW]
                                .rearrange("p s (h ch) -> p s h ch", ch=c.CH),
                            axis=AX.X)
                        logit = P_grp.tile([128, GRP * NH], f32, tag="lg")
                        nc.vector.scalar_tensor_tensor(
                            out=logit[:].rearrange("p (s h) -> p s h", h=NH),
                            in0=msrc[:].rearrange("p (s m) -> p s m", m=MB)
                                [:, :, FW:FW + NH],
                            scalar=0.5 * (1.0 + NEG),
                            in1=red[:].rearrange("p (s h) -> p s h", h=NH),
                            op0=ALU.mult, op1=ALU.add)
                        # per-edge exp weights written straight into the den
                        # cols of the scatter rhs (narrow, strided ACT out)
                        nc.scalar.activation(
                            out=wxw3[:].rearrange("p (s d) -> p s d", d=NDW)
                                [:, :, FW:FW + NH],
                            in_=logit[:].rearrange("p (s h) -> p s h", h=NH),
                            func=AF.Exp)
                        # weighted xs into group rhs buffer, batched per
                        # contiguous run within one gather tile
                        runs = []
                        for j in subs:
                            stream = 0 if j < c.TA else 1
                            s_str = (w * c.TA + j) if stream == 0 else (w * c.TB + j - c.TA)
                            k, t = divmod(s_str, c.CPC)
                            if ('no_gather' not in ablate and runs
                                    and runs[-1][0] == stream and runs[-1][1] == k
                                    and runs[-1][2] + runs[-1][3] == t):
                                runs[-1][3] += 1
                            else:
                                runs.append([stream, k, t, 1, j])
                        for stream, k, t0, nrun, j0 in runs:
                            jj0 = j0 - g0
                            if 'no_gather' in ablate:
                                gsl = xl1c[:, 0:nrun * gtab_w]
                            else:
                                gsl = gouts[(stream, k)][:, t0 * gtab_w:
                                                         (t0 + nrun) * gtab_w]
                            nc.vector.tensor_tensor(
                                out=wxw3[:, jj0 * NDW: (jj0 + nrun) * NDW]
                                    .rearrange("p (t d) -> p t d", d=NDW)[:, :, 0:FW]
                                    .rearrange("p t (h ch) -> p t h ch", ch=c.CH),
                                in0=gsl
                                    .rearrange("p (t d) -> p t d", d=gtab_w)[:, :, 0:FW]
                                    .rearrange("p t (h ch) -> p t h ch", ch=c.CH),
                                in1=wxw3[:, jj0 * NDW: (jj0 + nrun) * NDW]
                                    .rearrange("p (t d) -> p t d", d=NDW)
                                    [:, :, FW:FW + NH]
                                    .rearrange("p t (h o) -> p t h o", o=1)
                                    .to_broadcast([128, nrun, NH, c.CH]),
                                op=ALU.mult)
                        for j in subs:
                            jj = j - g0
                            if 'no_agg' in ablate and j != 0:
                                continue
                            nd_mm(oh_w[:, j * 128:(j + 1) * 128],
                                  wxw3[:, jj * NDW:(jj + 1) * NDW],
                                  stop=(j == c.T - 1))

                    payload = finalize_a(ps_nd, w)
                    if pend[0] is not None:
                        finalize_b(*pend[0])
                    pend[0] = (payload, w)
                finalize_b(*pend[0])

            with rep_loop():
                edge_layer(1)

            if skip_collective:
                nc.sync.dma_start(out=x2t[0:c.VPC, :], in_=x2shard[:, :])
            else:
                nc.gpsimd.collective_compute(
                    "AllGather", ALU.bypass,
                    ins=[x2shard[:].opt()], outs=[x2t[:].opt()],
                    replica_groups=[list(range(c.NC))])

            with rep_loop():
                edge_layer(2)

            pout = P_win.tile([64, 33], f32, tag="pout")
            nc.vector.tensor_copy(out=pout[:], in_=ps_pool[0:64, :])
            nc.sync.dma_start(out=partial[:, :], in_=pout[:])

    nc.compile()
    return nc


# ======================= host side =======================

def host_prep(inputs, c: Cfg):
    x = np.asarray(inputs['x'], np.float32)
    ei = np.asarray(inputs['edge_index'])
    ea = np.asarray(inputs['edge_attr'], np.float32)
    batch = np.asarray(inputs['batch'])
    src, dst = np.asarray(ei[0], np.int64), np.asarray(ei[1], np.int64)
    Wl1 = np.asarray(inputs['Wl1'], np.float32); Wr1 = np.asarray(inputs['Wr1'], np.float32)
    We1 = np.asarray(inputs['We1'], np.float32); att1 = np.asarray(inputs['att1'], np.float32)
    Wl2 = np.asarray(inputs['Wl2'], np.float32); Wr2 = np.asarray(inputs['Wr2'], np.float32)
    We2 = np.asarray(inputs['We2'], np.float32); att2 = np.asarray(inputs['att2'], np.float32)
    assert float(np.abs(np.asarray(inputs['b1'])).max()) == 0.0
    assert float(np.abs(np.asarray(inputs['b2'])).max()) == 0.0

    HEADS, CH, HC = c.HEADS, c.CH, c.HC
    att_bd = np.zeros((HC, HEADS), np.float32)
    for h in range(HEADS):
        att_bd[h * CH:(h + 1) * CH, h] = att1[h]
    a2 = att2.reshape(c.HID, 1)

    # self-loop attr: mean incoming edge_attr per node (0 for isolated nodes)
    N = c.N
    order = np.argsort(dst, kind='stable')
    ds = dst[order]; eas = ea[order]
    bounds = np.searchsorted(ds, np.arange(N))
    bsafe = np.minimum(bounds, len(ds) - 1)
    cnt = np.bincount(dst, minlength=N).astype(np.float32)
    sums = np.add.reduceat(eas, bsafe, axis=0)
    sums[cnt == 0] = 0.0
    loop_attr = sums / np.maximum(cnt, 1.0)[:, None]

    xTg = np.zeros((128, c.NPAD), bf16)
    xTg[:, :c.N] = x.T.astype(bf16)
    Wlr1x = np.concatenate([Wl1, Wl1 @ att_bd, Wr1, Wr1 @ att_bd], 1).astype(bf16)
    We1Q = np.concatenate([We1, We1 @ att_bd], 1).astype(bf16)
    We2Q = np.concatenate([We2, We2 @ a2], 1).astype(bf16)
    Wlr2x = np.concatenate([Wl2, Wl2 @ a2, Wr2, Wr2 @ a2], 1).astype(bf16)
    att1b = np.zeros((128, 396), bf16)
    for s in range(3):
        att1b[:, s * 132:s * 132 + 128] = att1.reshape(-1).astype(bf16)[None, :]
    att2b = np.zeros((128, 297), bf16)
    for s in range(9):
        att2b[:, s * 33:s * 33 + 32] = att2.reshape(-1).astype(bf16)[None, :]
    eye_ = np.eye(128, dtype=np.float32).astype(bf16)
    shared = dict(xT=xTg, Wlr1x=Wlr1x, We1Q=We1Q, We2Q=We2Q, Wlr2x=Wlr2x,
                  att1b=att1b, att2b=att2b,
                  eye_bf=eye_,
                  onescol=np.ones((128, 1), bf16),
                  iota64=np.tile(np.arange(64, dtype=np.float32)[None, :], (128, 1)))
    eye129 = np.vstack([np.eye(128, dtype=np.float32).astype(bf16),
                        np.zeros((1, 128), bf16)])

    in_maps = []
    for core in range(c.NC):
        c0 = core * c.VPCr
        m = (dst >= c0) & (dst < c0 + c.VPCr)
        s_c = src[m]; d_c = dst[m] - c0; e_c = ea[m]
        w_c = d_c >> 7
        half_c = (s_c >= c.HALF).astype(np.int64)

        E_slots = c.W * c.T * 128
        slot_src = np.zeros(E_slots, np.int64)
        slot_dl = np.full(E_slots, -1.0, np.float32)
        slot_ea = np.zeros((E_slots, 16), np.float32)
        order = np.lexsort((half_c, w_c))
        s_o, d_o, w_o, h_o, e_o = (s_c[order], d_c[order], w_c[order],
                                   half_c[order], e_c[order])
        nE = len(s_o)
        # group boundaries: edges sorted by (w, half)
        bounds = np.searchsorted(w_o * 2 + h_o, np.arange(c.W * 2 + 1))
        for w in range(c.W):
            for hh in (0, 1):
                lo, hi = bounds[w * 2 + hh], bounds[w * 2 + hh + 1]
                n = hi - lo
                lim = (c.TA if hh == 0 else c.TB) * 128
                assert n <= lim, (core, w, hh, n, lim)
                base = w * c.T * 128 + (0 if hh == 0 else c.TA * 128)
                slot_src[base:base + n] = s_o[lo:hi]
                slot_dl[base:base + n] = (d_o[lo:hi] - w * 128).astype(np.float32)
                slot_ea[base:base + n] = e_o[lo:hi]

        eaT_a = np.ascontiguousarray(slot_ea.T).astype(bf16)

        # one-hot scatter tables (host-precomputed)
        dl3i = slot_dl.reshape(c.W * c.T, 128).astype(np.int64)
        dl3i[dl3i < 0] = 128
        A = eye129[dl3i]                       # [S, p=edge, n] bf16
        ohW_a = np.ascontiguousarray(A.transpose(1, 0, 2)).reshape(128, -1)
        ohTW_a = np.ascontiguousarray(A.transpose(2, 0, 1)).reshape(128, -1)

        # self-loop mean-attr table, transposed per window
        la_core = np.zeros((c.VPC, 16), np.float32)
        la_core[:c.VPCr] = loop_attr[c0:c0 + c.VPCr]
        laTh_a = np.ascontiguousarray(la_core.T).astype(bf16)

        def build_idx(vals, ncall):
            out = np.zeros((128, ncall * (c.NIDX // 16)), np.int16)
            v = vals.reshape(ncall, c.NIDX)
            ii = np.arange(c.NIDX)
            for k in range(ncall):
                blk = np.zeros((16, c.NIDX // 16), np.int16)
                blk[ii % 16, ii // 16] = v[k].astype(np.int16)
                out[:, k * (c.NIDX // 16):(k + 1) * (c.NIDX // 16)] = np.tile(blk, (8, 1))
            return out

        slots3 = slot_src.reshape(c.W, c.T, 128)
        dl3 = slot_dl.reshape(c.W, c.T, 128)
        A_src = slots3[:, :c.TA, :].reshape(-1)
        B_src = slots3[:, c.TA:, :].reshape(-1)
        A_pad = dl3[:, :c.TA, :].reshape(-1) < 0
        B_pad = dl3[:, c.TA:, :].reshape(-1) < 0
        def r1(v):
            # tab1 storage row for node v (see stage-0 store coalescing)
            return (v >> 10) * 1024 + (v & 127) * 8 + ((v >> 7) & 7)
        i1A = np.where(A_pad, 0, r1(A_src))
        i1B = np.where(B_pad, 0, r1(B_src) - 24576)
        i2A = np.where(A_pad, 0, (A_src // c.VPCr) * c.VPC + (A_src % c.VPCr))
        i2B = np.where(B_pad, 0,
                       (B_src // c.VPCr) * c.VPC + (B_src % c.VPCr) - c.HALF2)
        for a in (i1A, i1B, i2A, i2B):
            assert a.min() >= 0 and a.max() < 32768

        xTloc_a = np.zeros((128, c.VPC), bf16)
        nreal = c.VPCr
        xTloc_a[:, :nreal] = x[c0:c0 + nreal].T.astype(bf16)

        blfull = np.full(c.VPC, -1.0, np.float32)
        blfull[:nreal] = np.asarray(batch[c0:c0 + nreal], np.float32)
        bl = blfull.reshape(c.W, 128).T.copy()

        im = dict(shared)
        im.update(xTloc=xTloc_a, eaT=eaT_a, ohW=ohW_a, ohTW=ohTW_a,
                  laTh=laTh_a,
                  idx1A=build_idx(i1A, c.NCALLA), idx1B=build_idx(i1B, c.NCALLB),
                  idx2A=build_idx(i2A, c.NCALLA), idx2B=build_idx(i2B, c.NCALLB),
                  batchloc=bl)
        in_maps.append(im)

    ctx = dict(Wc=np.asarray(inputs['Wc'], np.float32),
               bc=np.asarray(inputs['bc'], np.float32), G=c.G)
    return in_maps, ctx


def host_finalize(partials, ctx):
    tot = np.zeros((64, 33), np.float64)
    for p in partials:
        tot += np.asarray(p, np.float64)
    G = ctx['G']
    pooled = tot[:G, 0:32] / np.maximum(tot[:G, 32:33], 1.0)
    out = pooled.astype(np.float32) @ ctx['Wc'] + ctx['bc']
    return out.astype(np.float32)


# ======================= kernel entry =======================
_CACHE = {}


def _get_program(cfg_key, c):
    if cfg_key not in _CACHE:
        _CACHE[cfg_key] = build_program(c)
    return _CACHE[cfg_key]


def kernel(**inputs):
    """Full-input GATv2 kernel on 8 TRN2 NeuronCores. Returns [64, 2] f32."""
    from concourse import bass_utils

    ei = np.asarray(inputs['edge_index'])
    src = np.asarray(ei[0], np.int64)
    dst = np.asarray(ei[1], np.int64)
    N = int(np.asarray(inputs['x']).shape[0])
    NC = 8
    assert N % NC == 0, N
    VPCr = N // NC
    W = (VPCr + 127) // 128
    HALF = N // 2
    maxTA = maxTB = 1
    for core in range(NC):
        m = (dst >= core * VPCr) & (dst < (core + 1) * VPCr)
        w = (dst[m] - core * VPCr) >> 7
        hh = src[m] >= HALF
        cA = np.bincount(w[~hh], minlength=W)
        cB = np.bincount(w[hh], minlength=W)
        maxTA = max(maxTA, int(((cA + 127) // 128).max()))
        maxTB = max(maxTB, int(((cB + 127) // 128).max()))
    while (W * maxTA) % 7:
        maxTA += 1
    while (W * maxTB) % 7:
        maxTB += 1
    while (maxTA + maxTB) % 3:
        maxTB += 1

    c = Cfg(NC=NC, VPCr=VPCr, TA=maxTA, TB=maxTB, G=64)
    in_maps, ctx = host_prep(inputs, c)
    nc = _get_program((NC, VPCr, maxTA, maxTB), c)
    res = bass_utils.run_bass_kernel_spmd(nc, in_maps, core_ids=list(range(NC)))
    partials = [res.results[i]["partial"] for i in range(NC)]
    return host_finalize(partials, ctx)



# revision 39
# speedup vs baseline: 1.1051x; 1.0184x over previous
"""GATv2 (2-layer, 4+1 heads) TRN2 bass kernel, 8-core SPMD.

Accepts FULL inputs as produced by reference.setup_inputs() and returns the
FULL [64, 2] output. Internally: edges are partitioned by destination core
(6250 nodes/core) and destination 128-node window, padded to a static
(TA, TB) tile schedule; per-edge messages are assembled in PSUM (edge-attr
and one-hot-expanded x_r matmuls; gathered x_l joins via a third matmul in
layer 1 and a batched DVE add in layer 2); leaky-relu is evaluated as
0.6*linear + 0.4*|m| with the linear logit part riding extra matmul columns;
softmax runs max-free (logits are bounded); aggregation uses onehot-matmul
scatter into per-window PSUM accumulators, with the gather-free self-loop
subtile computed first so it stays off the window critical path, and the
1/den normalization + relu folded into per-head scalar-engine activations.
The one-hot scatter tables (oh / ohT) and the self-loop mean edge-attr table
are precomputed on the host and streamed from HBM; stage-0 stores coalesce
to 4KB descriptors and the A-stream gathers depend only on the A half of
tab1 so they start mid-stage-0. Layer-2 tables are exchanged with an
on-device AllGather. Per-core pooled partials [64, 33] are combined on the
host with the final [32x2] classifier.
"""
import sys
for _p in ('/opt/trn_rl_repo', '/root/.axon_site/_ro/trn_rl_repo'):
    if _p not in sys.path:
        sys.path.insert(0, _p)

import numpy as np
import ml_dtypes

import concourse.bass as bass
import concourse.bacc as bacc
import concourse.mybir as mybir
import concourse.tile as tile

bf16 = ml_dtypes.bfloat16
AF = mybir.ActivationFunctionType
ALU = mybir.AluOpType
AX = mybir.AxisListType
DT = mybir.dt
NEG = 0.2
EPS = 1e-16
L2_DVE_ADD = True
L1_DVE_ADD = True


class Cfg:
    def __init__(self, NC=8, VPCr=6250, TA=9, TB=9, G=64):
        self.NC = NC
        self.VPCr = VPCr
        self.N = NC * VPCr
        self.W = (VPCr + 127) // 128
        self.VPC = self.W * 128
        self.NPAD = NC * self.VPC
        self.TA, self.TB = TA, TB
        self.T = TA + TB
        self.G = G
        self.HALF = self.N // 2
        self.HALF2 = (NC // 2) * self.VPC
        self.HC = 128
        self.HEADS = 4
        self.CH = 32
        self.HID = 32
        self.SA = self.W * TA
        self.SB = self.W * TB
        self.CPC = max(d for d in (7, 3, 1)
                       if self.SA % d == 0 and self.SB % d == 0)
        self.NCALLA = self.SA // self.CPC
        self.NCALLB = self.SB // self.CPC
        self.NIDX = self.CPC * 128
        assert self.T % 3 == 0
        assert NC % 2 == 0 and VPCr % 2 == 0


def build_program(c: Cfg, debug=False, reps=1, skip_collective=False, ablate=()):
    nc = bacc.Bacc("TRN2", target_bir_lowering=False, debug=debug,
                   num_swdge_queues=4)
    f32, b16, i16 = DT.float32, DT.bfloat16, DT.int16

    def inp(name, shape, dt=f32):
        return nc.dram_tensor(name, shape, dt, kind="ExternalInput")

    xT = inp("xT", [128, c.NPAD], b16)
    xTloc = inp("xTloc", [128, c.VPC], b16)
    Wlr1x = inp("Wlr1x", [128, 264], b16)
    We1Q = inp("We1Q", [16, 132], b16)
    We2Q = inp("We2Q", [16, 33], b16)
    Wlr2x = inp("Wlr2x", [128, 66], b16)
    att1b = inp("att1b", [128, 396], b16)
    att2b = inp("att2b", [128, 297], b16)
    eye_bf = inp("eye_bf", [128, 128], b16)
    onescol = inp("onescol", [128, 1], b16)
    iota64 = inp("iota64", [128, 64], f32)
    eaT = inp("eaT", [16, c.W * c.T * 128], b16)
    ohW = inp("ohW", [128, c.W * c.T * 128], b16)
    ohTW = inp("ohTW", [128, c.W * c.T * 128], b16)
    laTh = inp("laTh", [16, c.W * 128], b16)
    NW = c.NIDX // 16
    idx1A = inp("idx1A", [128, c.NCALLA * NW], i16)
    idx1B = inp("idx1B", [128, c.NCALLB * NW], i16)
    idx2A = inp("idx2A", [128, c.NCALLA * NW], i16)
    idx2B = inp("idx2B", [128, c.NCALLB * NW], i16)
    batchloc = inp("batchloc", [128, c.W], f32)

    partial = nc.dram_tensor("partial", [64, 33], f32, kind="ExternalOutput")

    with tile.TileContext(nc) as tc:
        with (
            tc.tile_pool(name="const", bufs=1) as P_const,
            tc.tile_pool(name="res", bufs=1) as P_res,
            tc.tile_pool(name="s0", bufs=2) as P_s0,
            tc.tile_pool(name="gat", bufs=6) as P_gat,
            tc.tile_pool(name="ew", bufs=3) as P_ew,
            tc.tile_pool(name="mask", bufs=3) as P_mask,
            tc.tile_pool(name="grp", bufs=3) as P_grp,
            tc.tile_pool(name="win", bufs=2) as P_win,
            tc.tile_pool(name="pm", bufs=3, space="PSUM") as PS_m,
            tc.tile_pool(name="pnd", bufs=2, space="PSUM") as PS_nd,
            tc.tile_pool(name="pmisc", bufs=2, space="PSUM") as PS_misc,
            tc.tile_pool(name="ppool", bufs=1, space="PSUM") as PS_pool,
            tc.tile_pool(name="dram", bufs=1, space="DRAM") as P_dram,
        ):
            def load_const(t, shape, dt):
                s = P_const.tile(shape, dt, tag=t.name)
                nc.sync.dma_start(out=s[:], in_=t[:, :])
                return s

            Wlr1x_s = load_const(Wlr1x, [128, 264], b16)
            We1Q_s = load_const(We1Q, [16, 132], b16)
            We2Q_s = load_const(We2Q, [16, 33], b16)
            Wlr2x_s = load_const(Wlr2x, [128, 66], b16)
            att1b_s = load_const(att1b, [128, 396], b16)
            att2b_s = load_const(att2b, [128, 297], b16)
            eye_s = load_const(eye_bf, [128, 128], b16)
            onescol_s = load_const(onescol, [128, 1], b16)
            iota64_s = load_const(iota64, [128, 64], f32)
            laTh_s = load_const(laTh, [16, c.W * 128], b16)
            idx1A_s = load_const(idx1A, [128, c.NCALLA * NW], i16)
            idx1B_s = load_const(idx1B, [128, c.NCALLB * NW], i16)
            idx2A_s = load_const(idx2A, [128, c.NCALLA * NW], i16)
            idx2B_s = load_const(idx2B, [128, c.NCALLB * NW], i16)
            batchloc_s = load_const(batchloc, [128, c.W], f32)

            xl1c = P_res.tile([128, c.W * 132], b16)
            xr1c = P_res.tile([128, c.W * 132], b16)
            x2c = P_res.tile([128, c.W * 66], b16)

            tab1 = P_dram.tile([c.NPAD, 256], b16)
            x2shard = P_dram.tile([c.VPC, 128], b16)
            x2t = P_dram.tile([c.NC * c.VPC, 128], b16, addr_space="Shared")

            ps_pool = PS_pool.tile([128, 33], f32, tag="pool")

            # ================= stage 0 =================
            import contextlib
            def rep_loop():
                return tc.For_i(0, reps, 1) if reps > 1 else contextlib.nullcontext()
            NT = c.NPAD // 128
            assert NT % 8 == 0
            for n8 in range(0, NT, 8):
                nb = 8
                xt = P_s0.tile([128, 8 * 128], b16, tag="xt")
                nc.sync.dma_start(out=xt[:, 0:nb * 128],
                                  in_=xT[:, n8 * 128:(n8 + nb) * 128])
                row = P_s0.tile([128, 8 * 256], b16, tag="row")
                for q in range(nb):
                    ps = PS_m.tile([128, 264], f32, tag="m")
                    nc.tensor.matmul(ps[:], lhsT=xt[:, q * 128:(q + 1) * 128],
                                     rhs=Wlr1x_s[:], start=True, stop=True)
                    if q % 2:
                        nc.scalar.copy(out=row[:, q * 256:q * 256 + 256],
                                       in_=ps[:, 0:256])
                    else:
                        nc.vector.tensor_copy(out=row[:, q * 256:q * 256 + 256],
                                              in_=ps[:, 0:256])
                # tab1 row id = blk*1024 + p*8 + t: per-partition rows are
                # contiguous, so the store coalesces to one 4KB descriptor
                # per partition.
                nc.sync.dma_start(
                    out=tab1[n8 * 128:(n8 + nb) * 128, :]
                        .rearrange("(p t) d -> p t d", t=8),
                    in_=row[:, 0:nb * 256].rearrange("p (t d) -> p t d", d=256))
            WCH = 7
            assert c.W % WCH == 0
            for w0 in range(0, c.W, WCH):
                xt = P_s0.tile([128, WCH * 128], b16, tag="xtl")
                nc.sync.dma_start(out=xt[:],
                                  in_=xTloc[:, w0 * 128:(w0 + WCH) * 128])
                for w in range(w0, w0 + WCH):
                    q = w - w0
                    ps = PS_m.tile([128, 264], f32, tag="m")
                    nc.tensor.matmul(ps[:], lhsT=xt[:, q * 128:(q + 1) * 128],
                                     rhs=Wlr1x_s[:], start=True, stop=True)
                    nc.vector.tensor_copy(out=xl1c[:, w * 132:(w + 1) * 132],
                                          in_=ps[:, 0:132])
                    nc.scalar.copy(out=xr1c[:, w * 132:(w + 1) * 132],
                                   in_=ps[:, 132:264])

            # ================= edge layer sweep =================
            gq = [0]  # global gather-call counter for queue rotation
            gsems = [nc.alloc_semaphore(f"gsem{q}") for q in range(4)]
            for _gs in gsems:
                nc.gpsimd.sem_clear(_gs)
            def edge_layer(layer):
                L1 = layer == 1
                MB = 132 if L1 else 33
                FW = 128 if L1 else 32
                NH = 4 if L1 else 1
                GRP = 3 if L1 else (9 if c.T % 9 == 0 else 3)
                NDW = 132 if L1 else 33
                attb = att1b_s if L1 else att2b_s
                WeQ = We1Q_s if L1 else We2Q_s
                res_l = xl1c if L1 else x2c
                res_r = xr1c if L1 else x2c
                rl_w = 132 if L1 else 66
                rr_off = 0 if L1 else 33
                gtab_w = 256 if L1 else 128
                gidxA = idx1A_s if L1 else idx2A_s
                gidxB = idx1B_s if L1 else idx2B_s
                in_apA = tab1[0:c.HALF2 + 512, :] if L1 else x2t[:, :]
                in_apB = tab1[24576:, :] if L1 else x2t[c.HALF2:, :]

                gouts = {}

                def gather_call(stream, k):
                    gidx = gidxA if stream == 0 else gidxB
                    in_ap = in_apA if stream == 0 else in_apB
                    g = P_gat.tile([128, c.CPC * gtab_w], b16, tag=f"g{layer}{stream}")
                    nc.gpsimd.dma_gather(
                        out_ap=g[:].rearrange("p (t d) -> p t d", d=gtab_w),
                        in_ap=in_ap, idxs_ap=gidx[:, k * NW:(k + 1) * NW],
                        num_idxs=c.NIDX, num_idxs_reg=c.NIDX, elem_size=gtab_w,
                        queue_num=gq[0] % 4)
                    gq[0] += 1
                    gouts[(stream, k)] = g

                def xs_slice(stream, s):
                    if 'no_gather' in ablate:
                        return xl1c[:, 0:FW + NH]
                    k, t = divmod(s, c.CPC)
                    return gouts[(stream, k)][:, t * gtab_w: t * gtab_w + FW + NH]

                def issue_upto(w_ahead):
                    if 'no_gather' in ablate:
                        return
                    for stream, TX in ((0, c.TA), (1, c.TB)):
                        last_s = min(c.W, w_ahead + 1) * TX - 1
                        kmax = last_s // c.CPC
                        k0 = 0
                        while (stream, k0) in gouts:
                            k0 += 1
                        for k in range(k0, kmax + 1):
                            if (stream, k) not in gouts:
                                gather_call(stream, k)

                # ---- window finalize: normalize inline, PE tail pipelined ----
                def finalize_a(ps_nd, w):
                    dent = P_win.tile([128, NH], f32, tag="den")
                    nc.vector.tensor_scalar(out=dent[:], in0=ps_nd[:, FW:FW + NH],
                                            scalar1=EPS, scalar2=None, op0=ALU.add)
                    rcpd = P_win.tile([128, NH], f32, tag="rcp")
                    nc.vector.reciprocal(out=rcpd[:], in_=dent[:])
                    if L1:
                        h1r = P_win.tile([128, 128], b16, tag="h1r")
                        for h in range(NH):
                            nc.scalar.activation(
                                out=h1r[:, h * c.CH:(h + 1) * c.CH],
                                in_=ps_nd[:, h * c.CH:(h + 1) * c.CH],
                                func=AF.Relu, scale=rcpd[:, h:h + 1])
                        return h1r
                    else:
                        h2e = P_win.tile([128, 33], b16, tag="h2e")
                        nc.scalar.activation(out=h2e[:, 0:32], in_=ps_nd[:, 0:32],
                                             func=AF.Relu, scale=rcpd[:, 0:1])
                        nc.vector.tensor_copy(out=h2e[:, 32:33], in_=onescol_s[:])
                        ohB = P_win.tile([128, 64], b16, tag="ohB")
                        nc.vector.tensor_tensor(
                            out=ohB[:], in0=iota64_s[:],
                            in1=batchloc_s[:, w:w + 1].to_broadcast([128, 64]),
                            op=ALU.is_equal)
                        return (h2e, ohB)

                def finalize_b(payload, w):
                    if L1:
                        h1r = payload
                        ps_t2 = PS_misc.tile([128, 128], b16, tag="psmisc")
                        nc.tensor.transpose(ps_t2[:], h1r[:], eye_s[:])
                        h1T = P_win.tile([128, 128], b16, tag="h1T")
                        nc.scalar.copy(out=h1T[:], in_=ps_t2[:])
                        ps_x2 = PS_misc.tile([128, 66], f32, tag="psmisc")
                        nc.tensor.matmul(ps_x2[:], lhsT=h1T[:], rhs=Wlr2x_s[:],
                                         start=True, stop=True)
                        nc.vector.tensor_copy(out=x2c[:, w * 66:(w + 1) * 66],
                                              in_=ps_x2[:])
                        sh = P_win.tile([128, 66], b16, tag="sh")
                        nc.scalar.copy(out=sh[:], in_=ps_x2[:])
                        nc.scalar.dma_start(out=x2shard[w * 128:(w + 1) * 128, 0:66],
                                            in_=sh[:])
                    else:
                        h2e, ohB = payload
                        nc.tensor.matmul(ps_pool[0:64, :], lhsT=ohB[:], rhs=h2e[:],
                                         start=(w == 0), stop=(w == c.W - 1))

                pend = [None]

                for w in range(c.W):
                    issue_upto(w + 1)
                    eaT_w = P_ew.tile([16, c.T * 128], b16, tag="eaT")
                    nc.scalar.dma_start(out=eaT_w[:],
                                        in_=eaT[:, w * c.T * 128:(w + 1) * c.T * 128])
                    oh_w = P_ew.tile([128, c.T * 128], b16, tag="oh")
                    nc.sync.dma_start(out=oh_w[:],
                                      in_=ohW[:, w * c.T * 128:(w + 1) * c.T * 128])
                    ohT_w = P_ew.tile([128, c.T * 128], b16, tag="ohT")
                    nc.sync.dma_start(out=ohT_w[:],
                                      in_=ohTW[:, w * c.T * 128:(w + 1) * c.T * 128])

                    ps_nd = PS_nd.tile([128, NDW], f32, tag="nd")
                    first_mm = [True]

                    def nd_mm(lhsT, rhs, stop=False):
                        nc.tensor.matmul(ps_nd[:, 0:rhs.shape[1]], lhsT=lhsT, rhs=rhs,
                                         start=first_mm[0], stop=stop)
                        first_mm[0] = False

                    # ---- self subtile (accumulated into ps_nd via eye) ----
                    ps_s = PS_m.tile([128, MB], f32, tag="m")
                    nc.tensor.matmul(ps_s[:], lhsT=laTh_s[:, w * 128:(w + 1) * 128],
                                     rhs=WeQ[:], start=True, stop=False)
                    nc.tensor.matmul(ps_s[:], lhsT=eye_s[:],
                                     rhs=res_l[:, w * rl_w: w * rl_w + MB],
                                     start=False, stop=False)
                    nc.tensor.matmul(ps_s[:], lhsT=eye_s[:],
                                     rhs=res_r[:, w * rl_w + rr_off: w * rl_w + rr_off + MB],
                                     start=False, stop=True)
                    abs_ = P_win.tile([128, MB], b16, tag="sab")
                    nc.scalar.activation(out=abs_[:], in_=ps_s[:], func=AF.Abs,
                                         scale=(1.0 - NEG) / 2)
                    prs = P_win.tile([128, MB], b16, tag="spr")
                    nc.vector.tensor_tensor(out=prs[:], in0=abs_[:],
                                            in1=attb[:, 0:MB], op=ALU.mult)
                    reds = P_win.tile([128, NH], f32, tag="srd")
                    nc.vector.reduce_sum(
                        out=reds[:],
                        in_=prs[:, 0:FW].rearrange("p (h ch) -> p h ch", ch=c.CH),
                        axis=AX.X)
                    logs = P_win.tile([128, NH], f32, tag="slg")
                    nc.vector.scalar_tensor_tensor(
                        out=logs[:], in0=ps_s[:, FW:FW + NH],
                        scalar=0.5 * (1.0 + NEG), in1=reds[:],
                        op0=ALU.mult, op1=ALU.add)
                    wexps = P_win.tile([128, NH], b16, tag="swx")
                    nc.scalar.activation(out=wexps[:], in_=logs[:], func=AF.Exp)
                    wxws = P_win.tile([128, NDW], b16, tag="sww")
                    nc.vector.tensor_tensor(
                        out=wxws[:, 0:FW].rearrange("p (h ch) -> p h ch", ch=c.CH),
                        in0=res_l[:, w * rl_w: w * rl_w + FW]
                            .rearrange("p (h ch) -> p h ch", ch=c.CH),
                        in1=wexps[:].rearrange("p (h o) -> p h o", o=1)
                            .to_broadcast([128, NH, c.CH]),
                        op=ALU.mult)
                    nc.scalar.copy(out=wxws[:, FW:FW + NH], in_=wexps[:])
                    nd_mm(eye_s[:], wxws[:])
                    for g0 in range(0, c.T, GRP):
                        ps_m = PS_m.tile([128, GRP * MB], f32, tag="m")
                        subs = list(range(g0, g0 + GRP))
                        wxw3 = P_mask.tile([128, GRP * NDW], b16, tag="ww")
                        dve_add = L1_DVE_ADD if L1 else L2_DVE_ADD
                        if dve_add:
                            mbuf = P_grp.tile([128, GRP * MB], b16, tag="mb")
                        for j in subs:
                            jj = j - g0
                            stream = 0 if j < c.TA else 1
                            s_str = (w * c.TA + j) if stream == 0 else (w * c.TB + j - c.TA)
                            k, t = divmod(s_str, c.CPC)
                            if 'no_gather' not in ablate and (stream, k) not in gouts:
                                gather_call(stream, k)
                            if 'no_msgmm' in ablate:
                                continue
                            mb = ps_m[:, jj * MB:(jj + 1) * MB]
                            nc.tensor.matmul(mb, lhsT=eaT_w[:, j * 128:(j + 1) * 128],
                                             rhs=WeQ[:], start=True, stop=False)
                            if not dve_add:
                                nc.tensor.matmul(
                                    mb, lhsT=ohT_w[:, j * 128:(j + 1) * 128],
                                    rhs=res_r[:, w * rl_w + rr_off: w * rl_w + rr_off + MB],
                                    start=False, stop=False)
                                nc.tensor.matmul(mb, lhsT=eye_s[:],
                                                 rhs=xs_slice(stream, s_str),
                                                 start=False, stop=True)
                            else:
                                nc.tensor.matmul(
                                    mb, lhsT=ohT_w[:, j * 128:(j + 1) * 128],
                                    rhs=res_r[:, w * rl_w + rr_off: w * rl_w + rr_off + MB],
                                    start=False, stop=True)
                        if dve_add and 'no_msgmm' not in ablate:
                            aruns = []
                            for j in subs:
                                stream = 0 if j < c.TA else 1
                                s_str = (w * c.TA + j) if stream == 0 else (w * c.TB + j - c.TA)
                                k, t = divmod(s_str, c.CPC)
                                if (aruns and aruns[-1][0] == stream
                                        and aruns[-1][1] == k
                                        and aruns[-1][2] + aruns[-1][3] == t):
                                    aruns[-1][3] += 1
                                else:
                                    aruns.append([stream, k, t, 1, j])
                            for stream, k, t0, nrun, j0 in aruns:
                                jj0 = j0 - g0
                                nc.vector.tensor_tensor(
                                    out=mbuf[:, jj0 * MB:(jj0 + nrun) * MB]
                                        .rearrange("p (t d) -> p t d", d=MB),
                                    in0=ps_m[:, jj0 * MB:(jj0 + nrun) * MB]
                                        .rearrange("p (t d) -> p t d", d=MB),
                                    in1=gouts[(stream, k)]
                                        [:, t0 * gtab_w:(t0 + nrun) * gtab_w]
                                        .rearrange("p (t d) -> p t d", d=gtab_w)
                                        [:, :, 0:MB],
                                    op=ALU.add)
                        # group ACT/DVE chain
                        if 'no_groupchain' in ablate:
                            nc.vector.tensor_copy(out=wxw3[:, 0:GRP * NDW],
                                                  in_=att1b_s[:, 0:1].to_broadcast([128, GRP * NDW]))
                            for j in subs:
                                jj = j - g0
                                nd_mm(oh_w[:, j * 128:(j + 1) * 128],
                                      wxw3[:, jj * NDW:(jj + 1) * NDW],
                                      stop=(j == c.T - 1))
                            continue
                        msrc = mbuf if dve_add else ps_m
                        ab = P_grp.tile([128, GRP * MB], b16, tag="ab")
                        nc.scalar.activation(out=ab[:], in_=msrc[:], func=AF.Abs,
                                             scale=(1.0 - NEG) / 2)
                        prod = P_grp.tile([128, GRP * MB], b16, tag="pr")
                        nc.vector.tensor_tensor(out=prod[:], in0=ab[:],
                                                in1=attb[:, 0:GRP * MB], op=ALU.mult)
                        red = P_grp.tile([128, GRP * NH], f32, tag="rd")
                        nc.vector.reduce_sum(
                            out=red[:].rearrange("p (s h) -> p s h", h=NH),
                            in_=prod[:].rearrange("p (s m) -> p s m", m=MB)[:, :, 0:FW]
                                .rearrange("p s (h ch) -> p s h ch", ch=c.CH),
                            axis=AX.X)
                        logit = P_grp.tile([128, GRP * NH], f32, tag="lg")
                        nc.vector.scalar_tensor_tensor(
                            out=logit[:].rearrange("p (s h) -> p s h", h=NH),
                            in0=msrc[:].rearrange("p (s m) -> p s m", m=MB)
                                [:, :, FW:FW + NH],
                            scalar=0.5 * (1.0 + NEG),
                            in1=red[:].rearrange("p (s h) -> p s h", h=NH),
                            op0=ALU.mult, op1=ALU.add)
                        # per-edge exp weights written straight into the den
                        # cols of the scatter rhs (narrow, strided ACT out)
                        nc.scalar.activation(
                            out=wxw3[:].rearrange("p (s d) -> p s d", d=NDW)
                                [:, :, FW:FW + NH],
                            in_=logit[:].rearrange("p (s h) -> p s h", h=NH),
                            func=AF.Exp)
                        # weighted xs into group rhs buffer, batched per
                        # contiguous run within one gather tile
                        runs = []
                        for j in subs:
                            stream = 0 if j < c.TA else 1
                            s_str = (w * c.TA + j) if stream == 0 else (w * c.TB + j - c.TA)
                            k, t = divmod(s_str, c.CPC)
                            if ('no_gather' not in ablate and runs
                                    and runs[-1][0] == stream and runs[-1][1] == k
                                    and runs[-1][2] + runs[-1][3] == t):
                                runs[-1][3] += 1
                            else:
                                runs.append([stream, k, t, 1, j])
                        for stream, k, t0, nrun, j0 in runs:
                            jj0 = j0 - g0
                            if 'no_gather' in ablate:
                                gsl = xl1c[:, 0:nrun * gtab_w]
                            else:
                                gsl = gouts[(stream, k)][:, t0 * gtab_w:
                                                         (t0 + nrun) * gtab_w]
                            nc.vector.tensor_tensor(
                                out=wxw3[:, jj0 * NDW: (jj0 + nrun) * NDW]
                                    .rearrange("p (t d) -> p t d", d=NDW)[:, :, 0:FW]
                                    .rearrange("p t (h ch) -> p t h ch", ch=c.CH),
                                in0=gsl
                                    .rearrange("p (t d) -> p t d", d=gtab_w)[:, :, 0:FW]
                                    .rearrange("p t (h ch) -> p t h ch", ch=c.CH),
                                in1=wxw3[:, jj0 * NDW: (jj0 + nrun) * NDW]
                                    .rearrange("p (t d) -> p t d", d=NDW)
                                    [:, :, FW:FW + NH]
                                    .rearrange("p t (h o) -> p t h o", o=1)
                                    .to_broadcast([128, nrun, NH, c.CH]),
                                op=ALU.mult)
                        for j in subs:
                            jj = j - g0
                            if 'no_agg' in ablate and j != 0:
                                continue
                            nd_mm(oh_w[:, j * 128:(j + 1) * 128],
                                  wxw3[:, jj * NDW:(jj + 1) * NDW],
                                  stop=(j == c.T - 1))

                    payload = finalize_a(ps_nd, w)
                    if pend[0] is not None:
                        finalize_b(*pend[0])
                    pend[0] = (payload, w)
                finalize_b(*pend[0])

            with rep_loop():
                edge_layer(1)

            if skip_collective:
                nc.sync.dma_start(out=x2t[0:c.VPC, :], in_=x2shard[:, :])
            else:
                nc.gpsimd.collective_compute(
                    "AllGather", ALU.bypass,
                    ins=[x2shard[:].opt()], outs=[x2t[:].opt()],
                    replica_groups=[list(range(c.NC))])

            with rep_loop():
                edge_layer(2)

            pout = P_win.tile([64, 33], f32, tag="pout")
            nc.vector.tensor_copy(out=pout[:], in_=ps_pool[0:64, :])
            nc.sync.dma_start(out=partial[:, :], in_=pout[:])

    nc.compile()
    return nc


# ======================= host side =======================

def host_prep(inputs, c: Cfg):
    x = np.asarray(inputs['x'], np.float32)
    ei = np.asarray(inputs['edge_index'])
    ea = np.asarray(inputs['edge_attr'], np.float32)
    batch = np.asarray(inputs['batch'])
    src, dst = np.asarray(ei[0], np.int64), np.asarray(ei[1], np.int64)
    Wl1 = np.asarray(inputs['Wl1'], np.float32); Wr1 = np.asarray(inputs['Wr1'], np.float32)
    We1 = np.asarray(inputs['We1'], np.float32); att1 = np.asarray(inputs['att1'], np.float32)
    Wl2 = np.asarray(inputs['Wl2'], np.float32); Wr2 = np.asarray(inputs['Wr2'], np.float32)
    We2 = np.asarray(inputs['We2'], np.float32); att2 = np.asarray(inputs['att2'], np.float32)
    assert float(np.abs(np.asarray(inputs['b1'])).max()) == 0.0
    assert float(np.abs(np.asarray(inputs['b2'])).max()) == 0.0

    HEADS, CH, HC = c.HEADS, c.CH, c.HC
    att_bd = np.zeros((HC, HEADS), np.float32)
    for h in range(HEADS):
        att_bd[h * CH:(h + 1) * CH, h] = att1[h]
    a2 = att2.reshape(c.HID, 1)

    # self-loop attr: mean incoming edge_attr per node (0 for isolated nodes)
    N = c.N
    order = np.argsort(dst, kind='stable')
    ds = dst[order]; eas = ea[order]
    bounds = np.searchsorted(ds, np.arange(N))
    bsafe = np.minimum(bounds, len(ds) - 1)
    cnt = np.bincount(dst, minlength=N).astype(np.float32)
    sums = np.add.reduceat(eas, bsafe, axis=0)
    sums[cnt == 0] = 0.0
    loop_attr = sums / np.maximum(cnt, 1.0)[:, None]

    xTg = np.zeros((128, c.NPAD), bf16)
    xTg[:, :c.N] = x.T.astype(bf16)
    Wlr1x = np.concatenate([Wl1, Wl1 @ att_bd, Wr1, Wr1 @ att_bd], 1).astype(bf16)
    We1Q = np.concatenate([We1, We1 @ att_bd], 1).astype(bf16)
    We2Q = np.concatenate([We2, We2 @ a2], 1).astype(bf16)
    Wlr2x = np.concatenate([Wl2, Wl2 @ a2, Wr2, Wr2 @ a2], 1).astype(bf16)
    att1b = np.zeros((128, 396), bf16)
    for s in range(3):
        att1b[:, s * 132:s * 132 + 128] = att1.reshape(-1).astype(bf16)[None, :]
    att2b = np.zeros((128, 297), bf16)
    for s in range(9):
        att2b[:, s * 33:s * 33 + 32] = att2.reshape(-1).astype(bf16)[None, :]
    eye_ = np.eye(128, dtype=np.float32).astype(bf16)
    shared = dict(xT=xTg, Wlr1x=Wlr1x, We1Q=We1Q, We2Q=We2Q, Wlr2x=Wlr2x,
                  att1b=att1b, att2b=att2b,
                  eye_bf=eye_,
                  onescol=np.ones((128, 1), bf16),
                  iota64=np.tile(np.arange(64, dtype=np.float32)[None, :], (128, 1)))
    eye129 = np.vstack([np.eye(128, dtype=np.float32).astype(bf16),
                        np.zeros((1, 128), bf16)])

    in_maps = []
    for core in range(c.NC):
        c0 = core * c.VPCr
        m = (dst >= c0) & (dst < c0 + c.VPCr)
        s_c = src[m]; d_c = dst[m] - c0; e_c = ea[m]
        w_c = d_c >> 7
        half_c = (s_c >= c.HALF).astype(np.int64)

        E_slots = c.W * c.T * 128
        slot_src = np.zeros(E_slots, np.int64)
        slot_dl = np.full(E_slots, -1.0, np.float32)
        slot_ea = np.zeros((E_slots, 16), np.float32)
        order = np.lexsort((half_c, w_c))
        s_o, d_o, w_o, h_o, e_o = (s_c[order], d_c[order], w_c[order],
                                   half_c[order], e_c[order])
        nE = len(s_o)
        # group boundaries: edges sorted by (w, half)
        bounds = np.searchsorted(w_o * 2 + h_o, np.arange(c.W * 2 + 1))
        for w in range(c.W):
            for hh in (0, 1):
                lo, hi = bounds[w * 2 + hh], bounds[w * 2 + hh + 1]
                n = hi - lo
                lim = (c.TA if hh == 0 else c.TB) * 128
                assert n <= lim, (core, w, hh, n, lim)
                base = w * c.T * 128 + (0 if hh == 0 else c.TA * 128)
                slot_src[base:base + n] = s_o[lo:hi]
                slot_dl[base:base + n] = (d_o[lo:hi] - w * 128).astype(np.float32)
                slot_ea[base:base + n] = e_o[lo:hi]

        eaT_a = np.ascontiguousarray(slot_ea.T).astype(bf16)

        # one-hot scatter tables (host-precomputed)
        dl3i = slot_dl.reshape(c.W * c.T, 128).astype(np.int64)
        dl3i[dl3i < 0] = 128
        A = eye129[dl3i]                       # [S, p=edge, n] bf16
        ohW_a = np.ascontiguousarray(A.transpose(1, 0, 2)).reshape(128, -1)
        ohTW_a = np.ascontiguousarray(A.transpose(2, 0, 1)).reshape(128, -1)

        # self-loop mean-attr table, transposed per window
        la_core = np.zeros((c.VPC, 16), np.float32)
        la_core[:c.VPCr] = loop_attr[c0:c0 + c.VPCr]
        laTh_a = np.ascontiguousarray(la_core.T).astype(bf16)

        def build_idx(vals, ncall):
            out = np.zeros((128, ncall * (c.NIDX // 16)), np.int16)
            v = vals.reshape(ncall, c.NIDX)
            ii = np.arange(c.NIDX)
            for k in range(ncall):
                blk = np.zeros((16, c.NIDX // 16), np.int16)
                blk[ii % 16, ii // 16] = v[k].astype(np.int16)
                out[:, k * (c.NIDX // 16):(k + 1) * (c.NIDX // 16)] = np.tile(blk, (8, 1))
            return out

        slots3 = slot_src.reshape(c.W, c.T, 128)
        dl3 = slot_dl.reshape(c.W, c.T, 128)
        A_src = slots3[:, :c.TA, :].reshape(-1)
        B_src = slots3[:, c.TA:, :].reshape(-1)
        A_pad = dl3[:, :c.TA, :].reshape(-1) < 0
        B_pad = dl3[:, c.TA:, :].reshape(-1) < 0
        def r1(v):
            # tab1 storage row for node v (see stage-0 store coalescing)
            return (v >> 10) * 1024 + (v & 127) * 8 + ((v >> 7) & 7)
        i1A = np.where(A_pad, 0, r1(A_src))
        i1B = np.where(B_pad, 0, r1(B_src) - 24576)
        i2A = np.where(A_pad, 0, (A_src // c.VPCr) * c.VPC + (A_src % c.VPCr))
        i2B = np.where(B_pad, 0,
                       (B_src // c.VPCr) * c.VPC + (B_src % c.VPCr) - c.HALF2)
        for a in (i1A, i1B, i2A, i2B):
            assert a.min() >= 0 and a.max() < 32768

        xTloc_a = np.zeros((128, c.VPC), bf16)
        nreal = c.VPCr
        xTloc_a[:, :nreal] = x[c0:c0 + nreal].T.astype(bf16)

        blfull = np.full(c.VPC, -1.0, np.float32)
        blfull[:nreal] = np.asarray(batch[c0:c0 + nreal], np.float32)
        bl = blfull.reshape(c.W, 128).T.copy()

        im = dict(shared)
        im.update(xTloc=xTloc_a, eaT=eaT_a, ohW=ohW_a, ohTW=ohTW_a,
                  laTh=laTh_a,
                  idx1A=build_idx(i1A, c.NCALLA), idx1B=build_idx(i1B, c.NCALLB),
                  idx2A=build_idx(i2A, c.NCALLA), idx2B=build_idx(i2B, c.NCALLB),
                  batchloc=bl)
        in_maps.append(im)

    ctx = dict(Wc=np.asarray(inputs['Wc'], np.float32),
               bc=np.asarray(inputs['bc'], np.float32), G=c.G)
    return in_maps, ctx


def host_finalize(partials, ctx):
    tot = np.zeros((64, 33), np.float64)
    for p in partials:
        tot += np.asarray(p, np.float64)
    G = ctx['G']
    pooled = tot[:G, 0:32] / np.maximum(tot[:G, 32:33], 1.0)
    out = pooled.astype(np.float32) @ ctx['Wc'] + ctx['bc']
    return out.astype(np.float32)


# ======================= kernel entry =======================
_CACHE = {}


def _get_program(cfg_key, c):
    if cfg_key not in _CACHE:
        _CACHE[cfg_key] = build_program(c)
    return _CACHE[cfg_key]


def kernel(**inputs):
    """Full-input GATv2 kernel on 8 TRN2 NeuronCores. Returns [64, 2] f32."""
    from concourse import bass_utils

    ei = np.asarray(inputs['edge_index'])
    src = np.asarray(ei[0], np.int64)
    dst = np.asarray(ei[1], np.int64)
    N = int(np.asarray(inputs['x']).shape[0])
    NC = 8
    assert N % NC == 0, N
    VPCr = N // NC
    W = (VPCr + 127) // 128
    HALF = N // 2
    maxTA = maxTB = 1
    for core in range(NC):
        m = (dst >= core * VPCr) & (dst < (core + 1) * VPCr)
        w = (dst[m] - core * VPCr) >> 7
        hh = src[m] >= HALF
        cA = np.bincount(w[~hh], minlength=W)
        cB = np.bincount(w[hh], minlength=W)
        maxTA = max(maxTA, int(((cA + 127) // 128).max()))
        maxTB = max(maxTB, int(((cB + 127) // 128).max()))
    while (W * maxTA) % 7:
        maxTA += 1
    while (W * maxTB) % 7:
        maxTB += 1
    while (maxTA + maxTB) % 3:
        maxTB += 1

    c = Cfg(NC=NC, VPCr=VPCr, TA=maxTA, TB=maxTB, G=64)
    in_maps, ctx = host_prep(inputs, c)
    nc = _get_program((NC, VPCr, maxTA, maxTB), c)
    res = bass_utils.run_bass_kernel_spmd(nc, in_maps, core_ids=list(range(NC)))
    partials = [res.results[i]["partial"] for i in range(NC)]
    return host_finalize(partials, ctx)



# revision 40
# speedup vs baseline: 1.1225x; 1.0157x over previous
"""GATv2 (2-layer, 4+1 heads) TRN2 bass kernel, 8-core SPMD.

Accepts FULL inputs as produced by reference.setup_inputs() and returns the
FULL [64, 2] output. Internally: edges are partitioned by destination core
(6250 nodes/core) and destination 128-node window, padded to a static
(TA, TB) tile schedule; per-edge messages are assembled in PSUM (edge-attr
and one-hot-expanded x_r matmuls; gathered x_l joins via a third matmul in
layer 1 and a batched DVE add in layer 2); leaky-relu is evaluated as
0.6*linear + 0.4*|m| with the linear logit part riding extra matmul columns;
softmax runs max-free (logits are bounded); aggregation uses onehot-matmul
scatter into per-window PSUM accumulators, with the gather-free self-loop
subtile computed first so it stays off the window critical path, and the
1/den normalization + relu folded into per-head scalar-engine activations.
The one-hot scatter tables (oh / ohT) and the self-loop mean edge-attr table
are precomputed on the host and streamed from HBM; stage-0 stores coalesce
to 4KB descriptors and the A-stream gathers depend only on the A half of
tab1 so they start mid-stage-0. Layer-2 tables are exchanged with an
on-device AllGather. Per-core pooled partials [64, 33] are combined on the
host with the final [32x2] classifier.
"""
import sys
for _p in ('/opt/trn_rl_repo', '/root/.axon_site/_ro/trn_rl_repo'):
    if _p not in sys.path:
        sys.path.insert(0, _p)

import numpy as np
import ml_dtypes

import concourse.bass as bass
import concourse.bacc as bacc
import concourse.mybir as mybir
import concourse.tile as tile

bf16 = ml_dtypes.bfloat16
AF = mybir.ActivationFunctionType
ALU = mybir.AluOpType
AX = mybir.AxisListType
DT = mybir.dt
NEG = 0.2
EPS = 1e-16
L2_DVE_ADD = True
L1_DVE_ADD = True


class Cfg:
    def __init__(self, NC=8, VPCr=6250, TA=9, TB=9, G=64):
        self.NC = NC
        self.VPCr = VPCr
        self.N = NC * VPCr
        self.W = (VPCr + 127) // 128
        self.VPC = self.W * 128
        self.NPAD = NC * self.VPC
        self.TA, self.TB = TA, TB
        self.T = TA + TB
        self.G = G
        self.HALF = self.N // 2
        self.HALF2 = (NC // 2) * self.VPC
        self.HC = 128
        self.HEADS = 4
        self.CH = 32
        self.HID = 32
        self.SA = self.W * TA
        self.SB = self.W * TB
        self.CPC = max(d for d in (7, 3, 1)
                       if self.SA % d == 0 and self.SB % d == 0)
        self.NCALLA = self.SA // self.CPC
        self.NCALLB = self.SB // self.CPC
        self.NIDX = self.CPC * 128
        assert self.T % 3 == 0
        assert NC % 2 == 0 and VPCr % 2 == 0


def build_program(c: Cfg, debug=False, reps=1, skip_collective=False, ablate=()):
    nc = bacc.Bacc("TRN2", target_bir_lowering=False, debug=debug,
                   num_swdge_queues=4)
    f32, b16, i16 = DT.float32, DT.bfloat16, DT.int16

    def inp(name, shape, dt=f32):
        return nc.dram_tensor(name, shape, dt, kind="ExternalInput")

    xT = inp("xT", [128, c.NPAD], b16)
    xTloc = inp("xTloc", [128, c.VPC], b16)
    Wlr1x = inp("Wlr1x", [128, 264], b16)
    We1Q = inp("We1Q", [16, 132], b16)
    We2Q = inp("We2Q", [16, 33], b16)
    Wlr2x = inp("Wlr2x", [128, 66], b16)
    att1b = inp("att1b", [128, 396], b16)
    att2b = inp("att2b", [128, 297], b16)
    eye_bf = inp("eye_bf", [128, 128], b16)
    onescol = inp("onescol", [128, 1], b16)
    iota64 = inp("iota64", [128, 64], f32)
    eaT = inp("eaT", [16, c.W * c.T * 128], b16)
    ohW = inp("ohW", [128, c.W * c.T * 128], b16)
    ohTW = inp("ohTW", [128, c.W * c.T * 128], b16)
    laTh = inp("laTh", [16, c.W * 128], b16)
    NW = c.NIDX // 16
    idx1A = inp("idx1A", [128, c.NCALLA * NW], i16)
    idx1B = inp("idx1B", [128, c.NCALLB * NW], i16)
    idx2A = inp("idx2A", [128, c.NCALLA * NW], i16)
    idx2B = inp("idx2B", [128, c.NCALLB * NW], i16)
    batchloc = inp("batchloc", [128, c.W], f32)

    partial = nc.dram_tensor("partial", [64, 33], f32, kind="ExternalOutput")

    with tile.TileContext(nc) as tc:
        with (
            tc.tile_pool(name="const", bufs=1) as P_const,
            tc.tile_pool(name="res", bufs=1) as P_res,
            tc.tile_pool(name="s0", bufs=2) as P_s0,
            tc.tile_pool(name="gat", bufs=6) as P_gat,
            tc.tile_pool(name="ew", bufs=3) as P_ew,
            tc.tile_pool(name="mask", bufs=3) as P_mask,
            tc.tile_pool(name="grp", bufs=3) as P_grp,
            tc.tile_pool(name="win", bufs=2) as P_win,
            tc.tile_pool(name="pm", bufs=3, space="PSUM") as PS_m,
            tc.tile_pool(name="pnd", bufs=2, space="PSUM") as PS_nd,
            tc.tile_pool(name="pmisc", bufs=2, space="PSUM") as PS_misc,
            tc.tile_pool(name="ppool", bufs=1, space="PSUM") as PS_pool,
            tc.tile_pool(name="dram", bufs=1, space="DRAM") as P_dram,
        ):
            def load_const(t, shape, dt):
                s = P_const.tile(shape, dt, tag=t.name)
                nc.sync.dma_start(out=s[:], in_=t[:, :])
                return s

            Wlr1x_s = load_const(Wlr1x, [128, 264], b16)
            We1Q_s = load_const(We1Q, [16, 132], b16)
            We2Q_s = load_const(We2Q, [16, 33], b16)
            Wlr2x_s = load_const(Wlr2x, [128, 66], b16)
            att1b_s = load_const(att1b, [128, 396], b16)
            att2b_s = load_const(att2b, [128, 297], b16)
            eye_s = load_const(eye_bf, [128, 128], b16)
            onescol_s = load_const(onescol, [128, 1], b16)
            iota64_s = load_const(iota64, [128, 64], f32)
            laTh_s = load_const(laTh, [16, c.W * 128], b16)
            idx1A_s = load_const(idx1A, [128, c.NCALLA * NW], i16)
            idx1B_s = load_const(idx1B, [128, c.NCALLB * NW], i16)
            idx2A_s = load_const(idx2A, [128, c.NCALLA * NW], i16)
            idx2B_s = load_const(idx2B, [128, c.NCALLB * NW], i16)
            batchloc_s = load_const(batchloc, [128, c.W], f32)

            xl1c = P_res.tile([128, c.W * 132], b16)
            xr1c = P_res.tile([128, c.W * 132], b16)
            x2c = P_res.tile([128, c.W * 66], b16)

            tab1 = P_dram.tile([c.NPAD, 256], b16)
            x2shard = P_dram.tile([c.VPC, 128], b16)
            x2t = P_dram.tile([c.NC * c.VPC, 128], b16, addr_space="Shared")

            ps_pool = PS_pool.tile([128, 33], f32, tag="pool")

            # ================= stage 0 =================
            import contextlib
            def rep_loop():
                return tc.For_i(0, reps, 1) if reps > 1 else contextlib.nullcontext()
            NT = c.NPAD // 128
            assert NT % 8 == 0
            for n8 in range(0, NT, 8):
                nb = 8
                xt = P_s0.tile([128, 8 * 128], b16, tag="xt")
                nc.sync.dma_start(out=xt[:, 0:nb * 128],
                                  in_=xT[:, n8 * 128:(n8 + nb) * 128])
                row = P_s0.tile([128, 8 * 256], b16, tag="row")
                for q in range(nb):
                    ps = PS_m.tile([128, 264], f32, tag="m")
                    nc.tensor.matmul(ps[:], lhsT=xt[:, q * 128:(q + 1) * 128],
                                     rhs=Wlr1x_s[:], start=True, stop=True)
                    if q % 2:
                        nc.scalar.copy(out=row[:, q * 256:q * 256 + 256],
                                       in_=ps[:, 0:256])
                    else:
                        nc.vector.tensor_copy(out=row[:, q * 256:q * 256 + 256],
                                              in_=ps[:, 0:256])
                # tab1 row id = blk*1024 + p*8 + t: per-partition rows are
                # contiguous, so the store coalesces to one 4KB descriptor
                # per partition.
                nc.sync.dma_start(
                    out=tab1[n8 * 128:(n8 + nb) * 128, :]
                        .rearrange("(p t) d -> p t d", t=8),
                    in_=row[:, 0:nb * 256].rearrange("p (t d) -> p t d", d=256))
            WCH = 7
            assert c.W % WCH == 0
            for w0 in range(0, c.W, WCH):
                xt = P_s0.tile([128, WCH * 128], b16, tag="xtl")
                nc.sync.dma_start(out=xt[:],
                                  in_=xTloc[:, w0 * 128:(w0 + WCH) * 128])
                for w in range(w0, w0 + WCH):
                    q = w - w0
                    ps = PS_m.tile([128, 264], f32, tag="m")
                    nc.tensor.matmul(ps[:], lhsT=xt[:, q * 128:(q + 1) * 128],
                                     rhs=Wlr1x_s[:], start=True, stop=True)
                    nc.vector.tensor_copy(out=xl1c[:, w * 132:(w + 1) * 132],
                                          in_=ps[:, 0:132])
                    nc.scalar.copy(out=xr1c[:, w * 132:(w + 1) * 132],
                                   in_=ps[:, 132:264])

            # ================= edge layer sweep =================
            gq = [0]  # global gather-call counter for queue rotation
            gsems = [nc.alloc_semaphore(f"gsem{q}") for q in range(4)]
            for _gs in gsems:
                nc.gpsimd.sem_clear(_gs)
            def edge_layer(layer):
                L1 = layer == 1
                MB = 132 if L1 else 33
                FW = 128 if L1 else 32
                NH = 4 if L1 else 1
                GRP = 3 if L1 else (9 if c.T % 9 == 0 else 3)
                NDW = 132 if L1 else 33
                attb = att1b_s if L1 else att2b_s
                WeQ = We1Q_s if L1 else We2Q_s
                res_l = xl1c if L1 else x2c
                res_r = xr1c if L1 else x2c
                rl_w = 132 if L1 else 66
                rr_off = 0 if L1 else 33
                gtab_w = 256 if L1 else 128
                gidxA = idx1A_s if L1 else idx2A_s
                gidxB = idx1B_s if L1 else idx2B_s
                in_apA = tab1[0:c.HALF2 + 512, :] if L1 else x2t[:, :]
                in_apB = tab1[24576:, :] if L1 else x2t[c.HALF2:, :]

                gouts = {}

                def gather_call(stream, k):
                    gidx = gidxA if stream == 0 else gidxB
                    in_ap = in_apA if stream == 0 else in_apB
                    g = P_gat.tile([128, c.CPC * gtab_w], b16, tag=f"g{layer}{stream}")
                    nc.gpsimd.dma_gather(
                        out_ap=g[:].rearrange("p (t d) -> p t d", d=gtab_w),
                        in_ap=in_ap, idxs_ap=gidx[:, k * NW:(k + 1) * NW],
                        num_idxs=c.NIDX, num_idxs_reg=c.NIDX, elem_size=gtab_w,
                        queue_num=gq[0] % 4)
                    gq[0] += 1
                    gouts[(stream, k)] = g

                def xs_slice(stream, s):
                    if 'no_gather' in ablate:
                        return xl1c[:, 0:FW + NH]
                    k, t = divmod(s, c.CPC)
                    return gouts[(stream, k)][:, t * gtab_w: t * gtab_w + FW + NH]

                def issue_upto(w_ahead):
                    if 'no_gather' in ablate:
                        return
                    for stream, TX in ((0, c.TA), (1, c.TB)):
                        last_s = min(c.W, w_ahead + 1) * TX - 1
                        kmax = last_s // c.CPC
                        k0 = 0
                        while (stream, k0) in gouts:
                            k0 += 1
                        for k in range(k0, kmax + 1):
                            if (stream, k) not in gouts:
                                gather_call(stream, k)

                # ---- window finalize: normalize inline, PE tail pipelined ----
                def finalize_a(ps_nd, w):
                    dent = P_win.tile([128, NH], f32, tag="den")
                    nc.vector.tensor_scalar(out=dent[:], in0=ps_nd[:, FW:FW + NH],
                                            scalar1=EPS, scalar2=None, op0=ALU.add)
                    rcpd = P_win.tile([128, NH], f32, tag="rcp")
                    nc.vector.reciprocal(out=rcpd[:], in_=dent[:])
                    if L1:
                        h1r = P_win.tile([128, 128], b16, tag="h1r")
                        for h in range(NH):
                            nc.scalar.activation(
                                out=h1r[:, h * c.CH:(h + 1) * c.CH],
                                in_=ps_nd[:, h * c.CH:(h + 1) * c.CH],
                                func=AF.Relu, scale=rcpd[:, h:h + 1])
                        return h1r
                    else:
                        h2e = P_win.tile([128, 33], b16, tag="h2e")
                        nc.scalar.activation(out=h2e[:, 0:32], in_=ps_nd[:, 0:32],
                                             func=AF.Relu, scale=rcpd[:, 0:1])
                        nc.vector.tensor_copy(out=h2e[:, 32:33], in_=onescol_s[:])
                        ohB = P_win.tile([128, 64], b16, tag="ohB")
                        nc.vector.tensor_tensor(
                            out=ohB[:], in0=iota64_s[:],
                            in1=batchloc_s[:, w:w + 1].to_broadcast([128, 64]),
                            op=ALU.is_equal)
                        return (h2e, ohB)

                def finalize_b(payload, w):
                    if L1:
                        h1r = payload
                        ps_t2 = PS_misc.tile([128, 128], b16, tag="psmisc")
                        nc.tensor.transpose(ps_t2[:], h1r[:], eye_s[:])
                        h1T = P_win.tile([128, 128], b16, tag="h1T")
                        nc.scalar.copy(out=h1T[:], in_=ps_t2[:])
                        ps_x2 = PS_misc.tile([128, 66], f32, tag="psmisc")
                        nc.tensor.matmul(ps_x2[:], lhsT=h1T[:], rhs=Wlr2x_s[:],
                                         start=True, stop=True)
                        nc.vector.tensor_copy(out=x2c[:, w * 66:(w + 1) * 66],
                                              in_=ps_x2[:])
                        sh = P_win.tile([128, 66], b16, tag="sh")
                        nc.scalar.copy(out=sh[:], in_=ps_x2[:])
                        nc.scalar.dma_start(out=x2shard[w * 128:(w + 1) * 128, 0:66],
                                            in_=sh[:])
                    else:
                        h2e, ohB = payload
                        nc.tensor.matmul(ps_pool[0:64, :], lhsT=ohB[:], rhs=h2e[:],
                                         start=(w == 0), stop=(w == c.W - 1))

                pend = [None]

                for w in range(c.W):
                    issue_upto(w + 3)
                    eaT_w = P_ew.tile([16, c.T * 128], b16, tag="eaT")
                    nc.scalar.dma_start(out=eaT_w[:],
                                        in_=eaT[:, w * c.T * 128:(w + 1) * c.T * 128])
                    oh_w = P_ew.tile([128, c.T * 128], b16, tag="oh")
                    nc.sync.dma_start(out=oh_w[:],
                                      in_=ohW[:, w * c.T * 128:(w + 1) * c.T * 128])
                    ohT_w = P_ew.tile([128, c.T * 128], b16, tag="ohT")
                    nc.sync.dma_start(out=ohT_w[:],
                                      in_=ohTW[:, w * c.T * 128:(w + 1) * c.T * 128])

                    ps_nd = PS_nd.tile([128, NDW], f32, tag="nd")
                    first_mm = [True]

                    def nd_mm(lhsT, rhs, stop=False):
                        nc.tensor.matmul(ps_nd[:, 0:rhs.shape[1]], lhsT=lhsT, rhs=rhs,
                                         start=first_mm[0], stop=stop)
                        first_mm[0] = False

                    # ---- self subtile (accumulated into ps_nd via eye) ----
                    ps_s = PS_m.tile([128, MB], f32, tag="m")
                    nc.tensor.matmul(ps_s[:], lhsT=laTh_s[:, w * 128:(w + 1) * 128],
                                     rhs=WeQ[:], start=True, stop=False)
                    nc.tensor.matmul(ps_s[:], lhsT=eye_s[:],
                                     rhs=res_l[:, w * rl_w: w * rl_w + MB],
                                     start=False, stop=False)
                    nc.tensor.matmul(ps_s[:], lhsT=eye_s[:],
                                     rhs=res_r[:, w * rl_w + rr_off: w * rl_w + rr_off + MB],
                                     start=False, stop=True)
                    abs_ = P_win.tile([128, MB], b16, tag="sab")
                    nc.scalar.activation(out=abs_[:], in_=ps_s[:], func=AF.Abs,
                                         scale=(1.0 - NEG) / 2)
                    prs = P_win.tile([128, MB], b16, tag="spr")
                    nc.vector.tensor_tensor(out=prs[:], in0=abs_[:],
                                            in1=attb[:, 0:MB], op=ALU.mult)
                    reds = P_win.tile([128, NH], f32, tag="srd")
                    nc.vector.reduce_sum(
                        out=reds[:],
                        in_=prs[:, 0:FW].rearrange("p (h ch) -> p h ch", ch=c.CH),
                        axis=AX.X)
                    logs = P_win.tile([128, NH], f32, tag="slg")
                    nc.vector.scalar_tensor_tensor(
                        out=logs[:], in0=ps_s[:, FW:FW + NH],
                        scalar=0.5 * (1.0 + NEG), in1=reds[:],
                        op0=ALU.mult, op1=ALU.add)
                    wexps = P_win.tile([128, NH], b16, tag="swx")
                    nc.scalar.activation(out=wexps[:], in_=logs[:], func=AF.Exp)
                    wxws = P_win.tile([128, NDW], b16, tag="sww")
                    nc.vector.tensor_tensor(
                        out=wxws[:, 0:FW].rearrange("p (h ch) -> p h ch", ch=c.CH),
                        in0=res_l[:, w * rl_w: w * rl_w + FW]
                            .rearrange("p (h ch) -> p h ch", ch=c.CH),
                        in1=wexps[:].rearrange("p (h o) -> p h o", o=1)
                            .to_broadcast([128, NH, c.CH]),
                        op=ALU.mult)
                    nc.scalar.copy(out=wxws[:, FW:FW + NH], in_=wexps[:])
                    nd_mm(eye_s[:], wxws[:])
                    for g0 in range(0, c.T, GRP):
                        ps_m = PS_m.tile([128, GRP * MB], f32, tag="m")
                        subs = list(range(g0, g0 + GRP))
                        wxw3 = P_mask.tile([128, GRP * NDW], b16, tag="ww")
                        dve_add = L1_DVE_ADD if L1 else L2_DVE_ADD
                        if dve_add:
                            mbuf = P_grp.tile([128, GRP * MB], b16, tag="mb")
                        for j in subs:
                            jj = j - g0
                            stream = 0 if j < c.TA else 1
                            s_str = (w * c.TA + j) if stream == 0 else (w * c.TB + j - c.TA)
                            k, t = divmod(s_str, c.CPC)
                            if 'no_gather' not in ablate and (stream, k) not in gouts:
                                gather_call(stream, k)
                            if 'no_msgmm' in ablate:
                                continue
                            mb = ps_m[:, jj * MB:(jj + 1) * MB]
                            nc.tensor.matmul(mb, lhsT=eaT_w[:, j * 128:(j + 1) * 128],
                                             rhs=WeQ[:], start=True, stop=False)
                            if not dve_add:
                                nc.tensor.matmul(
                                    mb, lhsT=ohT_w[:, j * 128:(j + 1) * 128],
                                    rhs=res_r[:, w * rl_w + rr_off: w * rl_w + rr_off + MB],
                                    start=False, stop=False)
                                nc.tensor.matmul(mb, lhsT=eye_s[:],
                                                 rhs=xs_slice(stream, s_str),
                                                 start=False, stop=True)
                            else:
                                nc.tensor.matmul(
                                    mb, lhsT=ohT_w[:, j * 128:(j + 1) * 128],
                                    rhs=res_r[:, w * rl_w + rr_off: w * rl_w + rr_off + MB],
                                    start=False, stop=True)
                        if dve_add and 'no_msgmm' not in ablate:
                            aruns = []
                            for j in subs:
                                stream = 0 if j < c.TA else 1
                                s_str = (w * c.TA + j) if stream == 0 else (w * c.TB + j - c.TA)
                                k, t = divmod(s_str, c.CPC)
                                if (aruns and aruns[-1][0] == stream
                                        and aruns[-1][1] == k
                                        and aruns[-1][2] + aruns[-1][3] == t):
                                    aruns[-1][3] += 1
                                else:
                                    aruns.append([stream, k, t, 1, j])
                            for stream, k, t0, nrun, j0 in aruns:
                                jj0 = j0 - g0
                                nc.vector.tensor_tensor(
                                    out=mbuf[:, jj0 * MB:(jj0 + nrun) * MB]
                                        .rearrange("p (t d) -> p t d", d=MB),
                                    in0=ps_m[:, jj0 * MB:(jj0 + nrun) * MB]
                                        .rearrange("p (t d) -> p t d", d=MB),
                                    in1=gouts[(stream, k)]
                                        [:, t0 * gtab_w:(t0 + nrun) * gtab_w]
                                        .rearrange("p (t d) -> p t d", d=gtab_w)
                                        [:, :, 0:MB],
                                    op=ALU.add)
                        # group ACT/DVE chain
                        if 'no_groupchain' in ablate:
                            nc.vector.tensor_copy(out=wxw3[:, 0:GRP * NDW],
                                                  in_=att1b_s[:, 0:1].to_broadcast([128, GRP * NDW]))
                            for j in subs:
                                jj = j - g0
                                nd_mm(oh_w[:, j * 128:(j + 1) * 128],
                                      wxw3[:, jj * NDW:(jj + 1) * NDW],
                                      stop=(j == c.T - 1))
                            continue
                        msrc = mbuf if dve_add else ps_m
                        ab = P_grp.tile([128, GRP * MB], b16, tag="ab")
                        nc.scalar.activation(out=ab[:], in_=msrc[:], func=AF.Abs,
                                             scale=(1.0 - NEG) / 2)
                        prod = P_grp.tile([128, GRP * MB], b16, tag="pr")
                        nc.vector.tensor_tensor(out=prod[:], in0=ab[:],
                                                in1=attb[:, 0:GRP * MB], op=ALU.mult)
                        red = P_grp.tile([128, GRP * NH], f32, tag="rd")
                        nc.vector.reduce_sum(
                            out=red[:].rearrange("p (s h) -> p s h", h=NH),
                            in_=prod[:].rearrange("p (s m) -> p s m", m=MB)[:, :, 0:FW]
                                .rearrange("p s (h ch) -> p s h ch", ch=c.CH),
                            axis=AX.X)
                        logit = P_grp.tile([128, GRP * NH], f32, tag="lg")
                        nc.vector.scalar_tensor_tensor(
                            out=logit[:].rearrange("p (s h) -> p s h", h=NH),
                            in0=msrc[:].rearrange("p (s m) -> p s m", m=MB)
                                [:, :, FW:FW + NH],
                            scalar=0.5 * (1.0 + NEG),
                            in1=red[:].rearrange("p (s h) -> p s h", h=NH),
                            op0=ALU.mult, op1=ALU.add)
                        # per-edge exp weights written straight into the den
                        # cols of the scatter rhs (narrow, strided ACT out)
                        nc.scalar.activation(
                            out=wxw3[:].rearrange("p (s d) -> p s d", d=NDW)
                                [:, :, FW:FW + NH],
                            in_=logit[:].rearrange("p (s h) -> p s h", h=NH),
                            func=AF.Exp)
                        # weighted xs into group rhs buffer, batched per
                        # contiguous run within one gather tile
                        runs = []
                        for j in subs:
                            stream = 0 if j < c.TA else 1
                            s_str = (w * c.TA + j) if stream == 0 else (w * c.TB + j - c.TA)
                            k, t = divmod(s_str, c.CPC)
                            if ('no_gather' not in ablate and runs
                                    and runs[-1][0] == stream and runs[-1][1] == k
                                    and runs[-1][2] + runs[-1][3] == t):
                                runs[-1][3] += 1
                            else:
                                runs.append([stream, k, t, 1, j])
                        for stream, k, t0, nrun, j0 in runs:
                            jj0 = j0 - g0
                            if 'no_gather' in ablate:
                                gsl = xl1c[:, 0:nrun * gtab_w]
                            else:
                                gsl = gouts[(stream, k)][:, t0 * gtab_w:
                                                         (t0 + nrun) * gtab_w]
                            nc.vector.tensor_tensor(
                                out=wxw3[:, jj0 * NDW: (jj0 + nrun) * NDW]
                                    .rearrange("p (t d) -> p t d", d=NDW)[:, :, 0:FW]
                                    .rearrange("p t (h ch) -> p t h ch", ch=c.CH),
                                in0=gsl
                                    .rearrange("p (t d) -> p t d", d=gtab_w)[:, :, 0:FW]
                                    .rearrange("p t (h ch) -> p t h ch", ch=c.CH),
                                in1=wxw3[:, jj0 * NDW: (jj0 + nrun) * NDW]
                                    .rearrange("p (t d) -> p t d", d=NDW)
                                    [:, :, FW:FW + NH]
                                    .rearrange("p t (h o) -> p t h o", o=1)
                                    .to_broadcast([128, nrun, NH, c.CH]),
                                op=ALU.mult)
                        for j in subs:
                            jj = j - g0
                            if 'no_agg' in ablate and j != 0:
                                continue
                            nd_mm(oh_w[:, j * 128:(j + 1) * 128],
                                  wxw3[:, jj * NDW:(jj + 1) * NDW],
                                  stop=(j == c.T - 1))

                    payload = finalize_a(ps_nd, w)
                    if pend[0] is not None:
                        finalize_b(*pend[0])
                    pend[0] = (payload, w)
                finalize_b(*pend[0])

            with rep_loop():
                edge_layer(1)

            if skip_collective:
                nc.sync.dma_start(out=x2t[0:c.VPC, :], in_=x2shard[:, :])
            else:
                nc.gpsimd.collective_compute(
                    "AllGather", ALU.bypass,
                    ins=[x2shard[:].opt()], outs=[x2t[:].opt()],
                    replica_groups=[list(range(c.NC))])

            with rep_loop():
                edge_layer(2)

            pout = P_win.tile([64, 33], f32, tag="pout")
            nc.vector.tensor_copy(out=pout[:], in_=ps_pool[0:64, :])
            nc.sync.dma_start(out=partial[:, :], in_=pout[:])

    nc.compile()
    return nc


# ======================= host side =======================

def host_prep(inputs, c: Cfg):
    x = np.asarray(inputs['x'], np.float32)
    ei = np.asarray(inputs['edge_index'])
    ea = np.asarray(inputs['edge_attr'], np.float32)
    batch = np.asarray(inputs['batch'])
    src, dst = np.asarray(ei[0], np.int64), np.asarray(ei[1], np.int64)
    Wl1 = np.asarray(inputs['Wl1'], np.float32); Wr1 = np.asarray(inputs['Wr1'], np.float32)
    We1 = np.asarray(inputs['We1'], np.float32); att1 = np.asarray(inputs['att1'], np.float32)
    Wl2 = np.asarray(inputs['Wl2'], np.float32); Wr2 = np.asarray(inputs['Wr2'], np.float32)
    We2 = np.asarray(inputs['We2'], np.float32); att2 = np.asarray(inputs['att2'], np.float32)
    assert float(np.abs(np.asarray(inputs['b1'])).max()) == 0.0
    assert float(np.abs(np.asarray(inputs['b2'])).max()) == 0.0

    HEADS, CH, HC = c.HEADS, c.CH, c.HC
    att_bd = np.zeros((HC, HEADS), np.float32)
    for h in range(HEADS):
        att_bd[h * CH:(h + 1) * CH, h] = att1[h]
    a2 = att2.reshape(c.HID, 1)

    # self-loop attr: mean incoming edge_attr per node (0 for isolated nodes)
    N = c.N
    order = np.argsort(dst, kind='stable')
    ds = dst[order]; eas = ea[order]
    bounds = np.searchsorted(ds, np.arange(N))
    bsafe = np.minimum(bounds, len(ds) - 1)
    cnt = np.bincount(dst, minlength=N).astype(np.float32)
    sums = np.add.reduceat(eas, bsafe, axis=0)
    sums[cnt == 0] = 0.0
    loop_attr = sums / np.maximum(cnt, 1.0)[:, None]

    xTg = np.zeros((128, c.NPAD), bf16)
    xTg[:, :c.N] = x.T.astype(bf16)
    Wlr1x = np.concatenate([Wl1, Wl1 @ att_bd, Wr1, Wr1 @ att_bd], 1).astype(bf16)
    We1Q = np.concatenate([We1, We1 @ att_bd], 1).astype(bf16)
    We2Q = np.concatenate([We2, We2 @ a2], 1).astype(bf16)
    Wlr2x = np.concatenate([Wl2, Wl2 @ a2, Wr2, Wr2 @ a2], 1).astype(bf16)
    att1b = np.zeros((128, 396), bf16)
    for s in range(3):
        att1b[:, s * 132:s * 132 + 128] = att1.reshape(-1).astype(bf16)[None, :]
    att2b = np.zeros((128, 297), bf16)
    for s in range(9):
        att2b[:, s * 33:s * 33 + 32] = att2.reshape(-1).astype(bf16)[None, :]
    eye_ = np.eye(128, dtype=np.float32).astype(bf16)
    shared = dict(xT=xTg, Wlr1x=Wlr1x, We1Q=We1Q, We2Q=We2Q, Wlr2x=Wlr2x,
                  att1b=att1b, att2b=att2b,
                  eye_bf=eye_,
                  onescol=np.ones((128, 1), bf16),
                  iota64=np.tile(np.arange(64, dtype=np.float32)[None, :], (128, 1)))
    eye129 = np.vstack([np.eye(128, dtype=np.float32).astype(bf16),
                        np.zeros((1, 128), bf16)])

    in_maps = []
    for core in range(c.NC):
        c0 = core * c.VPCr
        m = (dst >= c0) & (dst < c0 + c.VPCr)
        s_c = src[m]; d_c = dst[m] - c0; e_c = ea[m]
        w_c = d_c >> 7
        half_c = (s_c >= c.HALF).astype(np.int64)

        E_slots = c.W * c.T * 128
        slot_src = np.zeros(E_slots, np.int64)
        slot_dl = np.full(E_slots, -1.0, np.float32)
        slot_ea = np.zeros((E_slots, 16), np.float32)
        order = np.lexsort((half_c, w_c))
        s_o, d_o, w_o, h_o, e_o = (s_c[order], d_c[order], w_c[order],
                                   half_c[order], e_c[order])
        nE = len(s_o)
        # group boundaries: edges sorted by (w, half)
        bounds = np.searchsorted(w_o * 2 + h_o, np.arange(c.W * 2 + 1))
        for w in range(c.W):
            for hh in (0, 1):
                lo, hi = bounds[w * 2 + hh], bounds[w * 2 + hh + 1]
                n = hi - lo
                lim = (c.TA if hh == 0 else c.TB) * 128
                assert n <= lim, (core, w, hh, n, lim)
                base = w * c.T * 128 + (0 if hh == 0 else c.TA * 128)
                slot_src[base:base + n] = s_o[lo:hi]
                slot_dl[base:base + n] = (d_o[lo:hi] - w * 128).astype(np.float32)
                slot_ea[base:base + n] = e_o[lo:hi]

        eaT_a = np.ascontiguousarray(slot_ea.T).astype(bf16)

        # one-hot scatter tables (host-precomputed)
        dl3i = slot_dl.reshape(c.W * c.T, 128).astype(np.int64)
        dl3i[dl3i < 0] = 128
        A = eye129[dl3i]                       # [S, p=edge, n] bf16
        ohW_a = np.ascontiguousarray(A.transpose(1, 0, 2)).reshape(128, -1)
        ohTW_a = np.ascontiguousarray(A.transpose(2, 0, 1)).reshape(128, -1)

        # self-loop mean-attr table, transposed per window
        la_core = np.zeros((c.VPC, 16), np.float32)
        la_core[:c.VPCr] = loop_attr[c0:c0 + c.VPCr]
        laTh_a = np.ascontiguousarray(la_core.T).astype(bf16)

        def build_idx(vals, ncall):
            out = np.zeros((128, ncall * (c.NIDX // 16)), np.int16)
            v = vals.reshape(ncall, c.NIDX)
            ii = np.arange(c.NIDX)
            for k in range(ncall):
                blk = np.zeros((16, c.NIDX // 16), np.int16)
                blk[ii % 16, ii // 16] = v[k].astype(np.int16)
                out[:, k * (c.NIDX // 16):(k + 1) * (c.NIDX // 16)] = np.tile(blk, (8, 1))
            return out

        slots3 = slot_src.reshape(c.W, c.T, 128)
        dl3 = slot_dl.reshape(c.W, c.T, 128)
        A_src = slots3[:, :c.TA, :].reshape(-1)
        B_src = slots3[:, c.TA:, :].reshape(-1)
        A_pad = dl3[:, :c.TA, :].reshape(-1) < 0
        B_pad = dl3[:, c.TA:, :].reshape(-1) < 0
        def r1(v):
            # tab1 storage row for node v (see stage-0 store coalescing)
            return (v >> 10) * 1024 + (v & 127) * 8 + ((v >> 7) & 7)
        i1A = np.where(A_pad, 0, r1(A_src))
        i1B = np.where(B_pad, 0, r1(B_src) - 24576)
        i2A = np.where(A_pad, 0, (A_src // c.VPCr) * c.VPC + (A_src % c.VPCr))
        i2B = np.where(B_pad, 0,
                       (B_src // c.VPCr) * c.VPC + (B_src % c.VPCr) - c.HALF2)
        for a in (i1A, i1B, i2A, i2B):
            assert a.min() >= 0 and a.max() < 32768

        xTloc_a = np.zeros((128, c.VPC), bf16)
        nreal = c.VPCr
        xTloc_a[:, :nreal] = x[c0:c0 + nreal].T.astype(bf16)

        blfull = np.full(c.VPC, -1.0, np.float32)
        blfull[:nreal] = np.asarray(batch[c0:c0 + nreal], np.float32)
        bl = blfull.reshape(c.W, 128).T.copy()

        im = dict(shared)
        im.update(xTloc=xTloc_a, eaT=eaT_a, ohW=ohW_a, ohTW=ohTW_a,
                  laTh=laTh_a,
                  idx1A=build_idx(i1A, c.NCALLA), idx1B=build_idx(i1B, c.NCALLB),
                  idx2A=build_idx(i2A, c.NCALLA), idx2B=build_idx(i2B, c.NCALLB),
                  batchloc=bl)
        in_maps.append(im)

    ctx = dict(Wc=np.asarray(inputs['Wc'], np.float32),
               bc=np.asarray(inputs['bc'], np.float32), G=c.G)
    return in_maps, ctx


def host_finalize(partials, ctx):
    tot = np.zeros((64, 33), np.float64)
    for p in partials:
        tot += np.asarray(p, np.float64)
    G = ctx['G']
    pooled = tot[:G, 0:32] / np.maximum(tot[:G, 32:33], 1.0)
    out = pooled.astype(np.float32) @ ctx['Wc'] + ctx['bc']
    return out.astype(np.float32)


# ======================= kernel entry =======================
_CACHE = {}


def _get_program(cfg_key, c):
    if cfg_key not in _CACHE:
        _CACHE[cfg_key] = build_program(c)
    return _CACHE[cfg_key]


def kernel(**inputs):
    """Full-input GATv2 kernel on 8 TRN2 NeuronCores. Returns [64, 2] f32."""
    from concourse import bass_utils

    ei = np.asarray(inputs['edge_index'])
    src = np.asarray(ei[0], np.int64)
    dst = np.asarray(ei[1], np.int64)
    N = int(np.asarray(inputs['x']).shape[0])
    NC = 8
    assert N % NC == 0, N
    VPCr = N // NC
    W = (VPCr + 127) // 128
    HALF = N // 2
    maxTA = maxTB = 1
    for core in range(NC):
        m = (dst >= core * VPCr) & (dst < (core + 1) * VPCr)
        w = (dst[m] - core * VPCr) >> 7
        hh = src[m] >= HALF
        cA = np.bincount(w[~hh], minlength=W)
        cB = np.bincount(w[hh], minlength=W)
        maxTA = max(maxTA, int(((cA + 127) // 128).max()))
        maxTB = max(maxTB, int(((cB + 127) // 128).max()))
    while (W * maxTA) % 7:
        maxTA += 1
    while (W * maxTB) % 7:
        maxTB += 1
    while (maxTA + maxTB) % 3:
        maxTB += 1

    c = Cfg(NC=NC, VPCr=VPCr, TA=maxTA, TB=maxTB, G=64)
    in_maps, ctx = host_prep(inputs, c)
    nc = _get_program((NC, VPCr, maxTA, maxTB), c)
    res = bass_utils.run_bass_kernel_spmd(nc, in_maps, core_ids=list(range(NC)))
    partials = [res.results[i]["partial"] for i in range(NC)]
    return host_finalize(partials, ctx)

